# revision 21
# baseline (speedup 1.0000x reference)
"""TRN2 Bass kernel for GQA MultiHeadAttention (B=2, S=2048, D=2048, 16 q-heads,
4 kv-heads, d_k=128) with QK-RMSNorm + interleaved RoPE + causal softmax + out-proj.

Sharding: 8 cores = (batch b in {0,1}) x (kv-head group g in {0..3}).
Each core computes its 4 q-heads' attention for its batch and a partial
out-projection y.T = Wo_g @ attn_out_g.T  [2048(e) x 2048(s)].
Host sums the 4 partials per batch (fp16) and adds bo / the bv term.

Key design points (v2):
- fp16 activations/weights end to end (same PE/DVE rate as bf16, 4x mantissa).
- No vector reciprocals anywhere: 1/x and 1/sqrt(x) computed on the scalar
  engine as exp(-ln x) / exp(-0.5 ln x); the whole program lives in the
  natural_log_exp activation table (exp/ln/identity/square) - zero mid-kernel
  ACT table loads.
- RMS-norm sums use replicated-column stationary matmuls (W[c,r]=1/w[c]^2),
  so the per-position sumsq lands broadcast across all 128 partitions and the
  normalization scale is applied as a plain elementwise multiply.
- K is pre-scaled by c0/rms_k in phase 1, so the phase-2 softmax exp uses
  immediate scale/bias (exp(s - 5); the -5 guards fp16 overflow and cancels
  in normalization).
- Softmax denominator: P tiles accumulated on DVE (fp16), then one
  ones-stationary matmul broadcasts the j-sum to all partitions; 1/l via
  ACT exp(-ln).
- RoPE half-swap done by DVE reads at shifted partition bases (no DMA).
- No softmax max-subtraction: RMSNorm bounds |score| <= sqrt(128)=11.3, and
  the -5 exp bias keeps exp(s-5) <= e^6.3 well inside fp16 range.
"""
import sys
import numpy as np

sys.path.insert(0, "/opt/trn_rl_repo")

import concourse.bass as bass  # noqa: E402
import concourse.tile as tile  # noqa: E402
from concourse import mybir  # noqa: E402
from concourse.bass_utils import run_bass_kernel_spmd  # noqa: E402

F32 = mybir.dt.float32
F16 = mybir.dt.float16
BF16 = mybir.dt.bfloat16
AF = mybir.ActivationFunctionType

P = 128
S = 2048
D = 2048
DK = 128
NH_LOC = 4          # q heads per core
NC = D // P         # 16 contraction chunks
NST = 4             # s-tiles of 512
STILE = 512
NJB = S // P        # 16 j/s blocks of 128
EPS = 1e-8
C0 = 1.0 / np.sqrt(DK)
EXPB = -5.0         # softmax exp bias (cancels in normalization)

USE_SHIFT = True    # DVE partition-base-shifted reads for the RoPE half swap
_F16 = np.float16


_NO_SPLIT_OPCODES = {"UnconditionalBranch", "Call", "RegisterMove", "EventSemaphore"}


def _split_excess_waits(nc):
    """Walrus codegen allows only 1 sync wait per instruction struct; Tile
    can emit more. Move excess waits onto same-engine NoOps inserted before."""
    import bass_rust
    counter = [0]
    for fn in nc.m.functions:
        for blk in fn.blocks:
            out = []
            changed = False
            for inst in blk.instructions:
                si = inst.sync_info
                limit = 1
                if (si is not None and len(si.on_wait) > limit
                        and inst.opcode not in _NO_SPLIT_OPCODES):
                    waits = list(si.on_wait)
                    for w in waits[:-limit]:
                        counter[0] += 1
                        nop = bass_rust.InstNoOp(
                            name=f"I-wsplit-{counter[0]}", ins=[], outs=[])
                        nop.engine = inst.engine
                        nop.sync_info = mybir.SyncInfo(on_wait=[w], on_update=[])
                        out.append(nop)
                    inst.sync_info = mybir.SyncInfo(
                        on_wait=waits[-limit:], on_update=list(si.on_update))
                    changed = True
                out.append(inst)
            if changed:
                blk.instructions = out
    return counter[0]


def _build_program(split_waits=True):
    nc = bass.Bass()

    def inp(name, shape, dt):
        return nc.declare_dram_parameter(name, list(shape), dt, isOutput=False)

    # all partition-major so every load is 128 big contiguous descriptors
    xt4 = inp("xt4", (NST, P, NC, STILE), F16)
    wq = inp("wq", (P, NC, NH_LOC * DK), F16)
    wk = inp("wk", (P, NC, DK), F16)
    wv = inp("wv", (P, NC, DK), F16)
    wo = inp("wo", (P, NH_LOC, D), F16)
    winvq = inp("winvq", (P, P), F16)     # replicated 1/w_q^2 columns
    winvk = inp("winvk", (P, P), F16)     # replicated 1/w_k^2 columns
    ones2d = inp("ones2d", (P, P), F16)
    taba = inp("taba", (P, S), F16)       # [cos; cos]
    tabb = inp("tabb", (P, S), F16)       # [sin; -sin] (shifted-base layout)
    maskt = inp("maskt", (P, NH_LOC, STILE), F16)
    bq = inp("bq", (P, NH_LOC), F32)
    bk = inp("bk", (P, 1), F32)
    yT = nc.declare_dram_parameter("yT", [D, S], F16, isOutput=True)

    from contextlib import ExitStack

    with tile.TileContext(nc) as tc, ExitStack() as top:
        const = top.enter_context(tc.tile_pool(name="const", bufs=1))

        wq_sb = const.tile([P, NC, NH_LOC * DK], F16, tag="wq")
        wk_sb = const.tile([P, NC, DK], F16, tag="wk")
        wv_sb = const.tile([P, NC, DK], F16, tag="wv")
        wo_sb = const.tile([P, NH_LOC, D], F16, tag="wo")
        winvq_sb = const.tile([P, P], F16, tag="winvq")
        winvk_sb = const.tile([P, P], F16, tag="winvk")
        ones_sb = const.tile([P, P], F16, tag="ones")
        taba_sb = const.tile([P, S], F16, tag="taba")
        tabb_sb = const.tile([P, S], F16, tag="tabb")
        mask_sb = const.tile([P, NH_LOC, STILE], F16, tag="mask")
        bq_sb = const.tile([P, NH_LOC], F32, tag="bq")
        bk_sb = const.tile([P, 1], F32, tag="bk")
        eps_sb = const.tile([P, 1], F32, tag="eps")
        nc.vector.memset(eps_sb[:], EPS)
        lnc0_sb = const.tile([P, 1], F32, tag="lnc0")
        nc.vector.memset(lnc0_sb[:], float(np.log(C0)))
        expb_sb = const.tile([P, 1], F32, tag="expb")
        nc.vector.memset(expb_sb[:], EXPB)
        tiny_sb = const.tile([P, 1], F32, tag="tiny")
        nc.vector.memset(tiny_sb[:], 1e-20)

        # persistent activation tensors
        qhat = const.tile([P, NH_LOC, S], F16, tag="qhat")   # [c, h, s]
        khat = const.tile([P, S], F16, tag="khat")           # [c, s], pre-scaled
        vsb = const.tile([P, NJB, DK], F16, tag="v")         # [s%128, block, c]
        aon = const.tile([P, NH_LOC, S], F16, tag="aon")     # [c, h, i]

        # ------- Phase 1: projections + RMS + RoPE, fused per (output, s-tile) -------
        with ExitStack() as ph1:
            xp = ph1.enter_context(tc.tile_pool(name="xp", bufs=2))
            t1p = ph1.enter_context(tc.tile_pool(name="t1p", bufs=6))
            rp = ph1.enter_context(tc.tile_pool(name="rp", bufs=4))
            ps1 = ph1.enter_context(tc.tile_pool(name="ps1", bufs=3, space="PSUM"))
            plsp = ph1.enter_context(tc.tile_pool(name="plsp", bufs=2, space="PSUM"))

            # startup-critical loads first: first wq + x chunk, then the rest bulk
            xt0 = xp.tile([P, NC, STILE], F16, tag="xt", name="xt0")
            nc.sync.dma_start(wq_sb[:, 0:1, :], wq[:, 0:1, :])
            nc.sync.dma_start(xt0[:, 0:1, :], xt4[0, :, 0:1, :])
            nc.sync.dma_start(wq_sb[:, 1:NC, :], wq[:, 1:NC, :])
            nc.sync.dma_start(xt0[:, 1:NC, :], xt4[0, :, 1:NC, :])
            nc.sync.dma_start(bq_sb[:], bq[:])
            nc.sync.dma_start(winvq_sb[:], winvq[:])
            nc.sync.dma_start(taba_sb[:], taba[:])
            nc.sync.dma_start(tabb_sb[:], tabb[:])
            # non-startup-critical bulk loads ride the idle gpsimd DGE queue
            nc.gpsimd.dma_start(wv_sb[:], wv[:])
            nc.gpsimd.dma_start(wk_sb[:], wk[:])
            nc.gpsimd.dma_start(winvk_sb[:], winvk[:])
            nc.gpsimd.dma_start(bk_sb[:], bk[:])
            nc.gpsimd.dma_start(ones_sb[:], ones2d[:])
            nc.gpsimd.dma_start(mask_sb[:], maskt[:])
            nc.gpsimd.dma_start(wo_sb[:], wo[:])

            for st in range(NST):
                if st == 0:
                    xt = xt0
                else:
                    xt = xp.tile([P, NC, STILE], F16, tag="xt")
                    nc.sync.dma_start(xt[:], xt4[st])
                ssl = bass.ts(st, STILE)

                for oi in list(range(NH_LOC)) + ["v", "k"]:
                    if oi == "v":
                        # v: output [s-block=128, c=128], 4 s-blocks per s-tile
                        ptv = ps1.tile([P, STILE], F32, tag="proj", name="ptv")
                        for sb in range(4):
                            for ch in range(NC):
                                nc.tensor.matmul(ptv[:, bass.ts(sb, DK)],
                                                 xt[:, ch, bass.ts(sb, P)], wv_sb[:, ch, :],
                                                 start=(ch == 0), stop=(ch == NC - 1))
                        for sb in range(4):
                            nc.vector.tensor_copy(vsb[:, st * 4 + sb, :], ptv[:, bass.ts(sb, DK)])
                        continue
                    is_q = oi != "k"
                    pt = ps1.tile([P, STILE], F32, tag="proj")
                    for ch in range(NC):
                        lw = wq_sb[:, ch, bass.ts(oi, DK)] if is_q else wk_sb[:, ch, :]
                        nc.tensor.matmul(pt[:], lw, xt[:, ch, :],
                                         start=(ch == 0), stop=(ch == NC - 1))
                    bias_ap = bq_sb[:, oi:oi + 1] if is_q else bk_sb[:, 0:1]
                    qf = t1p.tile([P, STILE], F16, tag="qf")
                    nc.scalar.activation(qf[:], pt[:], AF.Identity, bias=bias_ap)

                    # sumsq broadcast to all partitions via replicated stationary
                    sq = t1p.tile([P, STILE], F16, tag="sq")
                    nc.vector.tensor_mul(sq[:], qf[:], qf[:])
                    pls = plsp.tile([P, STILE], F32, tag="pls")
                    nc.tensor.matmul(pls[:], winvq_sb[:] if is_q else winvk_sb[:],
                                     sq[:], start=True, stop=True)
                    t1 = t1p.tile([P, STILE], F32, tag="t1")
                    nc.scalar.activation(t1[:], pls[:], AF.Ln,
                                         scale=1.0 / DK, bias=eps_sb[:, 0:1])
                    # q: 1/rms; k: c0/rms (bias = ln(c0))
                    rrb = t1p.tile([P, STILE], F16, tag="rrb")
                    if is_q:
                        nc.scalar.activation(rrb[:], t1[:], AF.Exp, scale=-0.5)
                    else:
                        nc.scalar.activation(rrb[:], t1[:], AF.Exp, scale=-0.5,
                                             bias=lnc0_sb[:, 0:1])

                    # RoPE on de-interleaved halves: rt = qf*[cos;cos] + shift(qf)*tabb
                    ta = rp.tile([P, STILE], F16, tag="ta")
                    nc.vector.tensor_mul(ta[:], qf[:], taba_sb[:, ssl])
                    tb = rp.tile([P, STILE], F16, tag="tb")
                    if USE_SHIFT:
                        nc.vector.tensor_mul(tb[0:64, :], qf[64:P, :], tabb_sb[64:P, ssl])
                        nc.vector.tensor_mul(tb[64:P, :], qf[0:64, :], tabb_sb[0:64, ssl])
                    else:
                        sw = rp.tile([P, STILE], F16, tag="sw")
                        nc.sync.dma_start(sw[0:64, :], qf[64:P, :])
                        nc.sync.dma_start(sw[64:P, :], qf[0:64, :])
                        nc.vector.tensor_mul(tb[0:64, :], sw[0:64, :], tabb_sb[64:P, ssl])
                        nc.vector.tensor_mul(tb[64:P, :], sw[64:P, :], tabb_sb[0:64, ssl])
                    rt = rp.tile([P, STILE], F16, tag="rt")
                    nc.vector.tensor_add(rt[:], ta[:], tb[:])
                    if is_q:
                        nc.vector.tensor_mul(qhat[:, oi, ssl], rt[:], rrb[:])
                    else:
                        nc.vector.tensor_mul(khat[:, ssl], rt[:], rrb[:])

        # ---------------- Phase 2: attention ----------------
        with ExitStack() as ph2:
            pp = ph2.enter_context(tc.tile_pool(name="pp", bufs=4))
            pap = ph2.enter_context(tc.tile_pool(name="pap", bufs=2))
            lp = ph2.enter_context(tc.tile_pool(name="lp", bufs=4))
            psst = ph2.enter_context(tc.tile_pool(name="psst", bufs=2, space="PSUM"))
            psao = ph2.enter_context(tc.tile_pool(name="psao", bufs=2, space="PSUM"))
            psl = ph2.enter_context(tc.tile_pool(name="psl", bufs=1, space="PSUM"))

            for it in range(NST):
                isl = bass.ts(it, STILE)
                njb = 4 * it + 4
                for hp in range(NH_LOC // 2):
                    ao2 = []
                    pacc2 = []
                    for h in (2 * hp, 2 * hp + 1):
                        ao_ps = psao.tile([P, STILE], F32, tag="ao")
                        pacc = pap.tile([P, STILE], F16, tag="pacc")
                        # jb blocks processed in pairs sharing one 2-bank psum
                        # tile and ONE exp instruction. Diagonal pairs use the
                        # earlier sub-block's column range for both subs (the
                        # extra columns are real scores, masked to 0 later).
                        for g in range(njb // 2):
                            jb0 = 2 * g
                            t0 = jb0 - 4 * it
                            lo = P * t0 if t0 > 0 else 0
                            csl = slice(lo, STILE)
                            i0 = it * STILE + lo
                            w = STILE - lo
                            st2 = psst.tile([P, 2, STILE], F32, tag="st")
                            for s in range(2):
                                nc.tensor.matmul(st2[:, s, csl],
                                                 khat[:, bass.ts(jb0 + s, P)],
                                                 qhat[:, h, bass.ds(i0, w)],
                                                 start=True, stop=True)
                            pt2 = pp.tile([P, 2, STILE], F16, tag="p")
                            nc.scalar.activation(pt2[:, :, csl], st2[:, :, csl],
                                                 AF.Exp, bias=expb_sb[:, 0:1])
                            for s in range(2):
                                jb = jb0 + s
                                t = jb - 4 * it
                                if t >= 0:
                                    nc.vector.tensor_mul(pt2[:, s, csl],
                                                         pt2[:, s, csl],
                                                         mask_sb[:, t, csl])
                                nc.tensor.matmul(ao_ps[:, csl], vsb[:, jb, :],
                                                 pt2[:, s, csl],
                                                 start=(jb == 0), stop=(jb == njb - 1))
                                if jb == 0:
                                    nc.vector.tensor_copy(pacc[:], pt2[:, 0, :])
                                else:
                                    nc.vector.tensor_add(pacc[:, csl], pacc[:, csl],
                                                         pt2[:, s, csl])
                        ao2.append(ao_ps)
                        pacc2.append(pacc)
                    # softmax denominators for the head pair in one ln/exp:
                    # broadcast j-sums, then 1/l = exp(-ln l)
                    pl = psl.tile([P, 2, STILE], F32, tag="pl")
                    for s in range(2):
                        nc.tensor.matmul(pl[:, s, :], ones_sb[:], pacc2[s][:],
                                         start=True, stop=True)
                    tl = lp.tile([P, 2, STILE], F32, tag="tl")
                    nc.scalar.activation(tl[:], pl[:], AF.Ln, bias=tiny_sb[:, 0:1])
                    rlb = lp.tile([P, 2, STILE], F32, tag="rlb")
                    nc.scalar.activation(rlb[:], tl[:], AF.Exp, scale=-1.0)
                    for s in range(2):
                        nc.vector.tensor_mul(aon[:, 2 * hp + s, isl], ao2[s][:],
                                             rlb[:, s, :])

        # ---------------- Phase 3: out-projection ----------------
        with ExitStack() as ph3:
            yp = ph3.enter_context(tc.tile_pool(name="yp", bufs=3))
            psy = ph3.enter_context(tc.tile_pool(name="psy", bufs=2, space="PSUM"))

            yT_v = yT.rearrange("(eb p) s -> eb p s", p=P)
            for eb in range(NJB):
                y_sb = yp.tile([P, S], F16, tag="ysb")
                # fc-outer so each wo stationary serves 4 moving matmuls
                yps = psy.tile([P, NST, STILE], F32, tag="y")
                for fc in range(NH_LOC):
                    for st in range(NST):
                        nc.tensor.matmul(yps[:, st, :], wo_sb[:, fc, bass.ts(eb, P)],
                                         aon[:, fc, bass.ts(st, STILE)],
                                         start=(fc == 0), stop=(fc == NH_LOC - 1))
                for st in range(NST):
                    if st % 2 == 0:
                        nc.vector.tensor_copy(y_sb[:, bass.ts(st, STILE)],
                                              yps[:, st, :])
                    else:
                        nc.scalar.activation(y_sb[:, bass.ts(st, STILE)],
                                             yps[:, st, :], AF.Copy)
                nc.sync.dma_start(yT_v[eb], y_sb[:])

    if split_waits:
        _split_excess_waits(nc)
    return nc


_PERM = np.concatenate([np.arange(0, DK, 2), np.arange(1, DK, 2)])  # de-interleave


def _prep_inputs(x, Wq, bq, Wk, bk, Wv, bv, Wo, bo, q_norm_w, k_norm_w):
    """Build the 8 per-core input maps. Core c -> (b = c // 4, g = c % 4)."""
    def f16(a):
        return np.ascontiguousarray(a, dtype=_F16)

    wq_p = q_norm_w[_PERM].astype(np.float32)
    wk_p = k_norm_w[_PERM].astype(np.float32)
    with np.errstate(divide="ignore"):
        winvq = np.where(wq_p != 0, 1.0 / np.maximum(wq_p * wq_p, 1e-30), 0.0)
        winvk = np.where(wk_p != 0, 1.0 / np.maximum(wk_p * wk_p, 1e-30), 0.0)

    inv_freq = 1.0 / (10000.0 ** (np.arange(0, DK, 2, dtype=np.float32) / np.float32(DK)))
    freqs = np.arange(S, dtype=np.float32)[:, None] * inv_freq[None, :]
    cosT = np.cos(freqs).T.astype(np.float32)  # [64, S]
    sinT = np.sin(freqs).T.astype(np.float32)
    taba = np.vstack([cosT, cosT])             # [128, S]
    tabb = np.vstack([sinT, -sinT])            # shifted-base layout

    pj = np.arange(P)[:, None, None]
    tt = np.arange(NH_LOC)[None, :, None]
    fi = np.arange(STILE)[None, None, :]
    maskt = ((P * tt + pj) <= fi).astype(np.float32)  # [128, 4, 512]

    xt4_b = []
    for b in range(2):
        xt = x[b].T.astype(np.float32)  # [d, s]
        xt4_b.append(f16(xt.reshape(NC, P, NST, STILE).transpose(2, 1, 0, 3)))

    winvq_rep = f16(np.tile(winvq[:, None], (1, P)))
    winvk_rep = f16(np.tile(winvk[:, None], (1, P)))

    in_maps = []
    for core in range(8):
        b, g = divmod(core, NH_LOC)
        hsl = slice(g * NH_LOC * DK, (g + 1) * NH_LOC * DK)
        ksl = slice(g * DK, (g + 1) * DK)

        wq_blk = Wq[hsl].astype(np.float32).copy()  # [512, d]
        # per-head de-interleave permutation + fold q_norm_w
        wq_blk = wq_blk.reshape(NH_LOC, DK, D)[:, _PERM, :] * wq_p[None, :, None]
        wq_t = wq_blk.reshape(NH_LOC * DK, D).T.reshape(NC, P, NH_LOC * DK)
        wq_t = wq_t.transpose(1, 0, 2)  # [P, NC, M]

        wk_blk = Wk[ksl].astype(np.float32)[_PERM, :] * wk_p[:, None]
        wk_t = wk_blk.T.reshape(NC, P, DK).transpose(1, 0, 2)
        wv_t = Wv[ksl].astype(np.float32).T.reshape(NC, P, DK).transpose(1, 0, 2)
        wo_t = Wo[:, hsl].astype(np.float32).T.reshape(NH_LOC, P, D).transpose(1, 0, 2)

        bq_blk = bq[hsl].astype(np.float32).reshape(NH_LOC, DK)[:, _PERM].T.copy()
        bk_blk = bk[ksl].astype(np.float32)[_PERM][:, None].copy()

        in_maps.append({
            "xt4": xt4_b[b],
            "wq": f16(wq_t), "wk": f16(wk_t), "wv": f16(wv_t), "wo": f16(wo_t),
            "winvq": winvq_rep, "winvk": winvk_rep,
            "ones2d": np.ones((P, P), _F16),
            "taba": f16(taba), "tabb": f16(tabb),
            "maskt": f16(maskt),
            "bq": np.ascontiguousarray(bq_blk), "bk": bk_blk,
        })
    return in_maps


_CACHED = {}


def _get_program():
    if "nc" not in _CACHED:
        _CACHED["nc"] = _build_program()
    return _CACHED["nc"]


def kernel(x, Wq, bq, Wk, bk, Wv, bv, Wo, bo, q_norm_w, k_norm_w, _trace=False, _tmpdir=None):
    x = np.asarray(x, np.float32)
    args = [np.asarray(a, np.float32) for a in
            (Wq, bq, Wk, bk, Wv, bv, Wo, bo, q_norm_w, k_norm_w)]
    Wq, bq, Wk, bk, Wv, bv, Wo, bo, q_norm_w, k_norm_w = args

    nc = _get_program()
    in_maps = _prep_inputs(x, Wq, bq, Wk, bk, Wv, bv, Wo, bo, q_norm_w, k_norm_w)
    res = run_bass_kernel_spmd(nc, in_maps, list(range(8)), trace=_trace, tmpdir=_tmpdir)

    out = np.zeros((2, S, D), np.float32)
    for core in range(8):
        b = core // 4
        out[b] += res.results[core]["yT"].astype(np.float32).T
    out += bo[None, None, :]
    # v-bias enters only via softmax-weighted average (weights sum to 1):
    if np.any(bv):
        out += (np.repeat(bv.reshape(4, DK), 4, axis=0).reshape(D) @ Wo.T)[None, None, :]
    kernel._last_result = res
    return out


# revision 25
# speedup vs baseline: 1.0383x; 1.0383x over previous
"""TRN2 Bass kernel for GQA MultiHeadAttention (B=2, S=2048, D=2048, 16 q-heads,
4 kv-heads, d_k=128) with QK-RMSNorm + interleaved RoPE + causal softmax + out-proj.

Sharding: 8 cores = (batch b in {0,1}) x (kv-head group g in {0..3}).
Each core computes its 4 q-heads' attention for its batch and a partial
out-projection y.T = Wo_g @ attn_out_g.T  [2048(e) x 2048(s)].
Host sums the 4 partials per batch (fp16) and adds bo / the bv term.

Key design points (v2):
- fp16 activations/weights end to end (same PE/DVE rate as bf16, 4x mantissa).
- No vector reciprocals anywhere: 1/x and 1/sqrt(x) computed on the scalar
  engine as exp(-ln x) / exp(-0.5 ln x); the whole program lives in the
  natural_log_exp activation table (exp/ln/identity/square) - zero mid-kernel
  ACT table loads.
- RMS-norm sums use replicated-column stationary matmuls (W[c,r]=1/w[c]^2),
  so the per-position sumsq lands broadcast across all 128 partitions and the
  normalization scale is applied as a plain elementwise multiply.
- K is pre-scaled by c0/rms_k in phase 1, so the phase-2 softmax exp uses
  immediate scale/bias (exp(s - 5); the -5 guards fp16 overflow and cancels
  in normalization).
- Softmax denominator: P tiles accumulated on DVE (fp16), then one
  ones-stationary matmul broadcasts the j-sum to all partitions; 1/l via
  ACT exp(-ln).
- RoPE half-swap done by DVE reads at shifted partition bases (no DMA).
- No softmax max-subtraction: RMSNorm bounds |score| <= sqrt(128)=11.3, and
  the -5 exp bias keeps exp(s-5) <= e^6.3 well inside fp16 range.
"""
import sys
import numpy as np

sys.path.insert(0, "/opt/trn_rl_repo")

import concourse.bass as bass  # noqa: E402
import concourse.tile as tile  # noqa: E402
from concourse import mybir  # noqa: E402
from concourse.bass_utils import run_bass_kernel_spmd  # noqa: E402

F32 = mybir.dt.float32
F16 = mybir.dt.float16
BF16 = mybir.dt.bfloat16
AF = mybir.ActivationFunctionType

P = 128
S = 2048
D = 2048
DK = 128
NH_LOC = 4          # q heads per core
NC = D // P         # 16 contraction chunks
NST = 4             # s-tiles of 512
STILE = 512
NJB = S // P        # 16 j/s blocks of 128
EPS = 1e-8
C0 = 1.0 / np.sqrt(DK)
EXPB = -5.0         # softmax exp bias (cancels in normalization)

USE_SHIFT = True    # DVE partition-base-shifted reads for the RoPE half swap
_F16 = np.float16


_NO_SPLIT_OPCODES = {"UnconditionalBranch", "Call", "RegisterMove", "EventSemaphore"}


def _split_excess_waits(nc):
    """Walrus codegen allows only 1 sync wait per instruction struct; Tile
    can emit more. Move excess waits onto same-engine NoOps inserted before."""
    import bass_rust
    counter = [0]
    for fn in nc.m.functions:
        for blk in fn.blocks:
            out = []
            changed = False
            for inst in blk.instructions:
                si = inst.sync_info
                limit = 1
                if (si is not None and len(si.on_wait) > limit
                        and inst.opcode not in _NO_SPLIT_OPCODES):
                    waits = list(si.on_wait)
                    for w in waits[:-limit]:
                        counter[0] += 1
                        nop = bass_rust.InstNoOp(
                            name=f"I-wsplit-{counter[0]}", ins=[], outs=[])
                        nop.engine = inst.engine
                        nop.sync_info = mybir.SyncInfo(on_wait=[w], on_update=[])
                        out.append(nop)
                    inst.sync_info = mybir.SyncInfo(
                        on_wait=waits[-limit:], on_update=list(si.on_update))
                    changed = True
                out.append(inst)
            if changed:
                blk.instructions = out
    return counter[0]


def _build_program(split_waits=True):
    nc = bass.Bass()

    def inp(name, shape, dt):
        return nc.declare_dram_parameter(name, list(shape), dt, isOutput=False)

    # all partition-major so every load is 128 big contiguous descriptors
    xt4 = inp("xt4", (NST, P, NC, STILE), F16)
    wq = inp("wq", (P, NC, NH_LOC * DK), F16)
    wk = inp("wk", (P, NC, DK), F16)
    wv = inp("wv", (P, NC, DK), F16)
    wo = inp("wo", (P, NH_LOC, D), F16)
    winvq = inp("winvq", (P, P), F16)     # replicated 1/w_q^2 columns
    winvk = inp("winvk", (P, P), F16)     # replicated 1/w_k^2 columns
    ones2d = inp("ones2d", (P, P), F16)
    taba = inp("taba", (P, S), F16)       # [cos; cos]
    tabb = inp("tabb", (P, S), F16)       # [sin; -sin] (shifted-base layout)
    maskt = inp("maskt", (P, NH_LOC, STILE), F16)
    bq = inp("bq", (P, NH_LOC), F32)
    bk = inp("bk", (P, 1), F32)
    yT = nc.declare_dram_parameter("yT", [D, S], F16, isOutput=True)

    from contextlib import ExitStack

    with tile.TileContext(nc) as tc, ExitStack() as top:
        const = top.enter_context(tc.tile_pool(name="const", bufs=1))

        wq_sb = const.tile([P, NC, NH_LOC * DK], F16, tag="wq")
        wk_sb = const.tile([P, NC, DK], F16, tag="wk")
        wv_sb = const.tile([P, NC, DK], F16, tag="wv")
        wo_sb = const.tile([P, NH_LOC, D], F16, tag="wo")
        winvq_sb = const.tile([P, P], F16, tag="winvq")
        winvk_sb = const.tile([P, P], F16, tag="winvk")
        ones_sb = const.tile([P, P], F16, tag="ones")
        taba_sb = const.tile([P, S], F16, tag="taba")
        tabb_sb = const.tile([P, S], F16, tag="tabb")
        mask_sb = const.tile([P, NH_LOC, STILE], F16, tag="mask")
        bq_sb = const.tile([P, NH_LOC], F32, tag="bq")
        bk_sb = const.tile([P, 1], F32, tag="bk")
        eps_sb = const.tile([P, 1], F32, tag="eps")
        nc.vector.memset(eps_sb[:], EPS)
        lnc0_sb = const.tile([P, 1], F32, tag="lnc0")
        nc.vector.memset(lnc0_sb[:], float(np.log(C0)))
        expb_sb = const.tile([P, 1], F32, tag="expb")
        nc.vector.memset(expb_sb[:], EXPB)
        tiny_sb = const.tile([P, 1], F32, tag="tiny")
        nc.vector.memset(tiny_sb[:], 1e-20)

        # persistent activation tensors
        qhat = const.tile([P, NH_LOC, S], F16, tag="qhat")   # [c, h, s]
        khat = const.tile([P, S], F16, tag="khat")           # [c, s], pre-scaled
        vsb = const.tile([P, NJB, DK], F16, tag="v")         # [s%128, block, c]
        aon = const.tile([P, NH_LOC, S], F16, tag="aon")     # [c, h, i]

        # ------- Phase 1: projections + RMS + RoPE, fused per (output, s-tile) -------
        with ExitStack() as ph1:
            xp = ph1.enter_context(tc.tile_pool(name="xp", bufs=2))
            t1p = ph1.enter_context(tc.tile_pool(name="t1p", bufs=6))
            rp = ph1.enter_context(tc.tile_pool(name="rp", bufs=4))
            ps1 = ph1.enter_context(tc.tile_pool(name="ps1", bufs=3, space="PSUM"))
            plsp = ph1.enter_context(tc.tile_pool(name="plsp", bufs=2, space="PSUM"))

            # startup-critical loads first: first wq + x chunk, then the rest bulk
            xt0 = xp.tile([P, NC, STILE], F16, tag="xt", name="xt0")
            nc.sync.dma_start(wq_sb[:, 0:1, :], wq[:, 0:1, :])
            nc.sync.dma_start(xt0[:, 0:1, :], xt4[0, :, 0:1, :])
            nc.sync.dma_start(wq_sb[:, 1:NC, :], wq[:, 1:NC, :])
            nc.sync.dma_start(xt0[:, 1:NC, :], xt4[0, :, 1:NC, :])
            nc.sync.dma_start(bq_sb[:], bq[:])
            nc.sync.dma_start(winvq_sb[:], winvq[:])
            nc.sync.dma_start(taba_sb[:], taba[:])
            nc.sync.dma_start(tabb_sb[:], tabb[:])
            # non-startup-critical bulk loads ride the idle gpsimd DGE queue.
            # The dummy copy reads xt0, so these transfers only start after
            # the startup-critical xt0 load is done and don't steal its
            # HBM bandwidth.
            dummy = const.tile([1, 1], F16, tag="dummy")
            nc.gpsimd.tensor_copy(dummy[:], xt0[0:1, NC - 1, 0:1])
            nc.gpsimd.dma_start(wv_sb[:], wv[:])
            nc.gpsimd.dma_start(wk_sb[:], wk[:])
            nc.gpsimd.dma_start(winvk_sb[:], winvk[:])
            nc.gpsimd.dma_start(bk_sb[:], bk[:])
            nc.gpsimd.dma_start(ones_sb[:], ones2d[:])
            nc.gpsimd.dma_start(mask_sb[:], maskt[:])
            nc.gpsimd.dma_start(wo_sb[:], wo[:])

            for st in range(NST):
                if st == 0:
                    xt = xt0
                else:
                    xt = xp.tile([P, NC, STILE], F16, tag="xt")
                    nc.sync.dma_start(xt[:], xt4[st])
                ssl = bass.ts(st, STILE)

                for oi in list(range(NH_LOC)) + ["v", "k"]:
                    if oi == "v":
                        # v: output [s-block=128, c=128], 4 s-blocks per s-tile
                        ptv = ps1.tile([P, STILE], F32, tag="proj", name="ptv")
                        for sb in range(4):
                            for ch in range(NC):
                                nc.tensor.matmul(ptv[:, bass.ts(sb, DK)],
                                                 xt[:, ch, bass.ts(sb, P)], wv_sb[:, ch, :],
                                                 start=(ch == 0), stop=(ch == NC - 1))
                        for sb in range(4):
                            nc.vector.tensor_copy(vsb[:, st * 4 + sb, :], ptv[:, bass.ts(sb, DK)])
                        continue
                    is_q = oi != "k"
                    pt = ps1.tile([P, STILE], F32, tag="proj")
                    for ch in range(NC):
                        lw = wq_sb[:, ch, bass.ts(oi, DK)] if is_q else wk_sb[:, ch, :]
                        nc.tensor.matmul(pt[:], lw, xt[:, ch, :],
                                         start=(ch == 0), stop=(ch == NC - 1))
                    bias_ap = bq_sb[:, oi:oi + 1] if is_q else bk_sb[:, 0:1]
                    qf = t1p.tile([P, STILE], F16, tag="qf")
                    nc.scalar.activation(qf[:], pt[:], AF.Identity, bias=bias_ap)

                    # sumsq broadcast to all partitions via replicated stationary
                    sq = t1p.tile([P, STILE], F16, tag="sq")
                    nc.vector.tensor_mul(sq[:], qf[:], qf[:])
                    pls = plsp.tile([P, STILE], F32, tag="pls")
                    nc.tensor.matmul(pls[:], winvq_sb[:] if is_q else winvk_sb[:],
                                     sq[:], start=True, stop=True)
                    t1 = t1p.tile([P, STILE], F32, tag="t1")
                    nc.scalar.activation(t1[:], pls[:], AF.Ln,
                                         scale=1.0 / DK, bias=eps_sb[:, 0:1])
                    # q: 1/rms; k: c0/rms (bias = ln(c0))
                    rrb = t1p.tile([P, STILE], F16, tag="rrb")
                    if is_q:
                        nc.scalar.activation(rrb[:], t1[:], AF.Exp, scale=-0.5)
                    else:
                        nc.scalar.activation(rrb[:], t1[:], AF.Exp, scale=-0.5,
                                             bias=lnc0_sb[:, 0:1])

                    # RoPE on de-interleaved halves: rt = qf*[cos;cos] + shift(qf)*tabb
                    ta = rp.tile([P, STILE], F16, tag="ta")
                    nc.vector.tensor_mul(ta[:], qf[:], taba_sb[:, ssl])
                    tb = rp.tile([P, STILE], F16, tag="tb")
                    if USE_SHIFT:
                        nc.vector.tensor_mul(tb[0:64, :], qf[64:P, :], tabb_sb[64:P, ssl])
                        nc.vector.tensor_mul(tb[64:P, :], qf[0:64, :], tabb_sb[0:64, ssl])
                    else:
                        sw = rp.tile([P, STILE], F16, tag="sw")
                        nc.sync.dma_start(sw[0:64, :], qf[64:P, :])
                        nc.sync.dma_start(sw[64:P, :], qf[0:64, :])
                        nc.vector.tensor_mul(tb[0:64, :], sw[0:64, :], tabb_sb[64:P, ssl])
                        nc.vector.tensor_mul(tb[64:P, :], sw[64:P, :], tabb_sb[0:64, ssl])
                    rt = rp.tile([P, STILE], F16, tag="rt")
                    nc.vector.tensor_add(rt[:], ta[:], tb[:])
                    if is_q:
                        nc.vector.tensor_mul(qhat[:, oi, ssl], rt[:], rrb[:])
                    else:
                        nc.vector.tensor_mul(khat[:, ssl], rt[:], rrb[:])

        # ---------------- Phase 2: attention ----------------
        with ExitStack() as ph2:
            pp = ph2.enter_context(tc.tile_pool(name="pp", bufs=4))
            pap = ph2.enter_context(tc.tile_pool(name="pap", bufs=4))
            lp = ph2.enter_context(tc.tile_pool(name="lp", bufs=4))
            psst = ph2.enter_context(tc.tile_pool(name="psst", bufs=2, space="PSUM"))
            psao = ph2.enter_context(tc.tile_pool(name="psao", bufs=2, space="PSUM"))
            psl = ph2.enter_context(tc.tile_pool(name="psl", bufs=2, space="PSUM"))

            for it in range(NST):
                isl = bass.ts(it, STILE)
                njb = 4 * it + 4
                for h in range(NH_LOC):
                    ao_ps = psao.tile([P, STILE], F32, tag="ao")
                    # two independent accumulation chains (even/odd sub-block)
                    # halve the serial DVE dependency depth
                    pacc_e = pap.tile([P, STILE], F16, tag="pacc_e")
                    pacc_o = pap.tile([P, STILE], F16, tag="pacc_o")
                    # jb blocks processed in pairs sharing one 2-bank psum
                    # tile and ONE exp instruction. Diagonal pairs use the
                    # earlier sub-block's column range for both subs (the
                    # extra columns are real scores, masked to 0 later).
                    for g in range(njb // 2):
                        jb0 = 2 * g
                        t0 = jb0 - 4 * it
                        lo = P * t0 if t0 > 0 else 0
                        csl = slice(lo, STILE)
                        i0 = it * STILE + lo
                        w = STILE - lo
                        st2 = psst.tile([P, 2, STILE], F32, tag="st")
                        for s in range(2):
                            nc.tensor.matmul(st2[:, s, csl],
                                             khat[:, bass.ts(jb0 + s, P)],
                                             qhat[:, h, bass.ds(i0, w)],
                                             start=True, stop=True)
                        pt2 = pp.tile([P, 2, STILE], F16, tag="p")
                        nc.scalar.activation(pt2[:, :, csl], st2[:, :, csl],
                                             AF.Exp, bias=expb_sb[:, 0:1])
                        for s in range(2):
                            jb = jb0 + s
                            t = jb - 4 * it
                            if t >= 0:
                                nc.vector.tensor_mul(pt2[:, s, csl], pt2[:, s, csl],
                                                     mask_sb[:, t, csl])
                            nc.tensor.matmul(ao_ps[:, csl], vsb[:, jb, :],
                                             pt2[:, s, csl],
                                             start=(jb == 0), stop=(jb == njb - 1))
                            pacc = pacc_e if s == 0 else pacc_o
                            if g == 0:
                                nc.vector.tensor_copy(pacc[:], pt2[:, s, :])
                            else:
                                nc.vector.tensor_add(pacc[:, csl], pacc[:, csl],
                                                     pt2[:, s, csl])
                    nc.vector.tensor_add(pacc_e[:], pacc_e[:], pacc_o[:])
                    # softmax denominator: broadcast j-sum, then 1/l = exp(-ln l)
                    pl = psl.tile([P, STILE], F32, tag="pl")
                    nc.tensor.matmul(pl[:], ones_sb[:], pacc_e[:], start=True,
                                     stop=True)
                    tl = lp.tile([P, STILE], F32, tag="tl")
                    nc.scalar.activation(tl[:], pl[:], AF.Ln, bias=tiny_sb[:, 0:1])
                    rlb = lp.tile([P, STILE], F32, tag="rlb")
                    nc.scalar.activation(rlb[:], tl[:], AF.Exp, scale=-1.0)
                    nc.vector.tensor_mul(aon[:, h, isl], ao_ps[:], rlb[:])

        # ---------------- Phase 3: out-projection ----------------
        with ExitStack() as ph3:
            yp = ph3.enter_context(tc.tile_pool(name="yp", bufs=3))
            psy = ph3.enter_context(tc.tile_pool(name="psy", bufs=2, space="PSUM"))

            yT_v = yT.rearrange("(eb p) s -> eb p s", p=P)
            for eb in range(NJB):
                y_sb = yp.tile([P, S], F16, tag="ysb")
                # fc-outer so each wo stationary serves 4 moving matmuls
                yps = psy.tile([P, NST, STILE], F32, tag="y")
                for fc in range(NH_LOC):
                    for st in range(NST):
                        nc.tensor.matmul(yps[:, st, :], wo_sb[:, fc, bass.ts(eb, P)],
                                         aon[:, fc, bass.ts(st, STILE)],
                                         start=(fc == 0), stop=(fc == NH_LOC - 1))
                for st in range(NST):
                    if st % 2 == 0:
                        nc.vector.tensor_copy(y_sb[:, bass.ts(st, STILE)],
                                              yps[:, st, :])
                    else:
                        nc.scalar.activation(y_sb[:, bass.ts(st, STILE)],
                                             yps[:, st, :], AF.Copy)
                nc.sync.dma_start(yT_v[eb], y_sb[:])

    if split_waits:
        _split_excess_waits(nc)
    return nc


_PERM = np.concatenate([np.arange(0, DK, 2), np.arange(1, DK, 2)])  # de-interleave


def _prep_inputs(x, Wq, bq, Wk, bk, Wv, bv, Wo, bo, q_norm_w, k_norm_w):
    """Build the 8 per-core input maps. Core c -> (b = c // 4, g = c % 4)."""
    def f16(a):
        return np.ascontiguousarray(a, dtype=_F16)

    wq_p = q_norm_w[_PERM].astype(np.float32)
    wk_p = k_norm_w[_PERM].astype(np.float32)
    with np.errstate(divide="ignore"):
        winvq = np.where(wq_p != 0, 1.0 / np.maximum(wq_p * wq_p, 1e-30), 0.0)
        winvk = np.where(wk_p != 0, 1.0 / np.maximum(wk_p * wk_p, 1e-30), 0.0)

    inv_freq = 1.0 / (10000.0 ** (np.arange(0, DK, 2, dtype=np.float32) / np.float32(DK)))
    freqs = np.arange(S, dtype=np.float32)[:, None] * inv_freq[None, :]
    cosT = np.cos(freqs).T.astype(np.float32)  # [64, S]
    sinT = np.sin(freqs).T.astype(np.float32)
    taba = np.vstack([cosT, cosT])             # [128, S]
    tabb = np.vstack([sinT, -sinT])            # shifted-base layout

    pj = np.arange(P)[:, None, None]
    tt = np.arange(NH_LOC)[None, :, None]
    fi = np.arange(STILE)[None, None, :]
    maskt = ((P * tt + pj) <= fi).astype(np.float32)  # [128, 4, 512]

    xt4_b = []
    for b in range(2):
        xt = x[b].T.astype(np.float32)  # [d, s]
        xt4_b.append(f16(xt.reshape(NC, P, NST, STILE).transpose(2, 1, 0, 3)))

    winvq_rep = f16(np.tile(winvq[:, None], (1, P)))
    winvk_rep = f16(np.tile(winvk[:, None], (1, P)))

    in_maps = []
    for core in range(8):
        b, g = divmod(core, NH_LOC)
        hsl = slice(g * NH_LOC * DK, (g + 1) * NH_LOC * DK)
        ksl = slice(g * DK, (g + 1) * DK)

        wq_blk = Wq[hsl].astype(np.float32).copy()  # [512, d]
        # per-head de-interleave permutation + fold q_norm_w
        wq_blk = wq_blk.reshape(NH_LOC, DK, D)[:, _PERM, :] * wq_p[None, :, None]
        wq_t = wq_blk.reshape(NH_LOC * DK, D).T.reshape(NC, P, NH_LOC * DK)
        wq_t = wq_t.transpose(1, 0, 2)  # [P, NC, M]

        wk_blk = Wk[ksl].astype(np.float32)[_PERM, :] * wk_p[:, None]
        wk_t = wk_blk.T.reshape(NC, P, DK).transpose(1, 0, 2)
        wv_t = Wv[ksl].astype(np.float32).T.reshape(NC, P, DK).transpose(1, 0, 2)
        wo_t = Wo[:, hsl].astype(np.float32).T.reshape(NH_LOC, P, D).transpose(1, 0, 2)

        bq_blk = bq[hsl].astype(np.float32).reshape(NH_LOC, DK)[:, _PERM].T.copy()
        bk_blk = bk[ksl].astype(np.float32)[_PERM][:, None].copy()

        in_maps.append({
            "xt4": xt4_b[b],
            "wq": f16(wq_t), "wk": f16(wk_t), "wv": f16(wv_t), "wo": f16(wo_t),
            "winvq": winvq_rep, "winvk": winvk_rep,
            "ones2d": np.ones((P, P), _F16),
            "taba": f16(taba), "tabb": f16(tabb),
            "maskt": f16(maskt),
            "bq": np.ascontiguousarray(bq_blk), "bk": bk_blk,
        })
    return in_maps


_CACHED = {}


def _get_program():
    if "nc" not in _CACHED:
        _CACHED["nc"] = _build_program()
    return _CACHED["nc"]


def kernel(x, Wq, bq, Wk, bk, Wv, bv, Wo, bo, q_norm_w, k_norm_w, _trace=False, _tmpdir=None):
    x = np.asarray(x, np.float32)
    args = [np.asarray(a, np.float32) for a in
            (Wq, bq, Wk, bk, Wv, bv, Wo, bo, q_norm_w, k_norm_w)]
    Wq, bq, Wk, bk, Wv, bv, Wo, bo, q_norm_w, k_norm_w = args

    nc = _get_program()
    in_maps = _prep_inputs(x, Wq, bq, Wk, bk, Wv, bv, Wo, bo, q_norm_w, k_norm_w)
    res = run_bass_kernel_spmd(nc, in_maps, list(range(8)), trace=_trace, tmpdir=_tmpdir)

    out = np.zeros((2, S, D), np.float32)
    for core in range(8):
        b = core // 4
        out[b] += res.results[core]["yT"].astype(np.float32).T
    out += bo[None, None, :]
    # v-bias enters only via softmax-weighted average (weights sum to 1):
    if np.any(bv):
        out += (np.repeat(bv.reshape(4, DK), 4, axis=0).reshape(D) @ Wo.T)[None, None, :]
    kernel._last_result = res
    return out


# revision 28
# speedup vs baseline: 1.1175x; 1.0762x over previous
"""TRN2 Bass kernel for GQA MultiHeadAttention (B=2, S=2048, D=2048, 16 q-heads,
4 kv-heads, d_k=128) with QK-RMSNorm + interleaved RoPE + causal softmax + out-proj.

Sharding: 8 cores = (batch b in {0,1}) x (kv-head group g in {0..3}).
Each core computes its 4 q-heads' attention for its batch and a partial
out-projection y.T = Wo_g @ attn_out_g.T  [2048(e) x 2048(s)].
Host sums the 4 partials per batch (fp16) and adds bo / the bv term.

Key design points (v2):
- fp16 activations/weights end to end (same PE/DVE rate as bf16, 4x mantissa).
- No vector reciprocals anywhere: 1/x and 1/sqrt(x) computed on the scalar
  engine as exp(-ln x) / exp(-0.5 ln x); the whole program lives in the
  natural_log_exp activation table (exp/ln/identity/square) - zero mid-kernel
  ACT table loads.
- RMS-norm sums use replicated-column stationary matmuls (W[c,r]=1/w[c]^2),
  so the per-position sumsq lands broadcast across all 128 partitions and the
  normalization scale is applied as a plain elementwise multiply.
- K is pre-scaled by c0/rms_k in phase 1, so the phase-2 softmax exp uses
  immediate scale/bias (exp(s - 5); the -5 guards fp16 overflow and cancels
  in normalization).
- Softmax denominator: P tiles accumulated on DVE (fp16), then one
  ones-stationary matmul broadcasts the j-sum to all partitions; 1/l via
  ACT exp(-ln).
- RoPE half-swap done by DVE reads at shifted partition bases (no DMA).
- No softmax max-subtraction: RMSNorm bounds |score| <= sqrt(128)=11.3, and
  the -5 exp bias keeps exp(s-5) <= e^6.3 well inside fp16 range.
"""
import sys
import numpy as np

sys.path.insert(0, "/opt/trn_rl_repo")

import concourse.bass as bass  # noqa: E402
import concourse.tile as tile  # noqa: E402
from concourse import mybir  # noqa: E402
from concourse.bass_utils import run_bass_kernel_spmd  # noqa: E402

F32 = mybir.dt.float32
F16 = mybir.dt.float16
BF16 = mybir.dt.bfloat16
AF = mybir.ActivationFunctionType

P = 128
S = 2048
D = 2048
DK = 128
NH_LOC = 4          # q heads per core
NC = D // P         # 16 contraction chunks
NST = 4             # s-tiles of 512
STILE = 512
NJB = S // P        # 16 j/s blocks of 128
EPS = 1e-8
C0 = 1.0 / np.sqrt(DK)
EXPB = -5.0         # softmax exp bias (cancels in normalization)

USE_SHIFT = True    # DVE partition-base-shifted reads for the RoPE half swap
_F16 = np.float16


_NO_SPLIT_OPCODES = {"UnconditionalBranch", "Call", "RegisterMove", "EventSemaphore"}


def _split_excess_waits(nc):
    """Walrus codegen allows only 1 sync wait per instruction struct; Tile
    can emit more. Move excess waits onto same-engine NoOps inserted before."""
    import bass_rust
    counter = [0]
    for fn in nc.m.functions:
        for blk in fn.blocks:
            out = []
            changed = False
            for inst in blk.instructions:
                si = inst.sync_info
                limit = 1
                if (si is not None and len(si.on_wait) > limit
                        and inst.opcode not in _NO_SPLIT_OPCODES):
                    waits = list(si.on_wait)
                    for w in waits[:-limit]:
                        counter[0] += 1
                        nop = bass_rust.InstNoOp(
                            name=f"I-wsplit-{counter[0]}", ins=[], outs=[])
                        nop.engine = inst.engine
                        nop.sync_info = mybir.SyncInfo(on_wait=[w], on_update=[])
                        out.append(nop)
                    inst.sync_info = mybir.SyncInfo(
                        on_wait=waits[-limit:], on_update=list(si.on_update))
                    changed = True
                out.append(inst)
            if changed:
                blk.instructions = out
    return counter[0]


def _build_program(split_waits=True):
    nc = bass.Bass()

    def inp(name, shape, dt):
        return nc.declare_dram_parameter(name, list(shape), dt, isOutput=False)

    # all partition-major so every load is 128 big contiguous descriptors
    xt4 = inp("xt4", (NST, P, NC, STILE), F16)
    wq = inp("wq", (P, NC, NH_LOC * DK), F16)
    wk = inp("wk", (P, NC, DK), F16)
    wv = inp("wv", (P, NC, DK), F16)
    wo = inp("wo", (P, NH_LOC, D), F16)
    winvq = inp("winvq", (P, P), F16)     # replicated 1/w_q^2 columns
    winvk = inp("winvk", (P, P), F16)     # replicated 1/w_k^2 columns
    ones2d = inp("ones2d", (P, P), F16)
    taba = inp("taba", (P, S), F16)       # [cos; cos]
    tabb = inp("tabb", (P, S), F16)       # [sin; -sin] (shifted-base layout)
    maskt = inp("maskt", (P, NH_LOC, STILE), F16)
    bq = inp("bq", (P, NH_LOC), F32)
    bk = inp("bk", (P, 1), F32)
    yT = nc.declare_dram_parameter("yT", [D, S], F16, isOutput=True)

    from contextlib import ExitStack

    with tile.TileContext(nc) as tc, ExitStack() as top:
        const = top.enter_context(tc.tile_pool(name="const", bufs=1))

        wq_sb = const.tile([P, NC, NH_LOC * DK], F16, tag="wq")
        wk_sb = const.tile([P, NC, DK], F16, tag="wk")
        wv_sb = const.tile([P, NC, DK], F16, tag="wv")
        wo_sb = const.tile([P, NH_LOC, D], F16, tag="wo")
        winvq_sb = const.tile([P, P], F16, tag="winvq")
        winvk_sb = const.tile([P, P], F16, tag="winvk")
        ones_sb = const.tile([P, P], F16, tag="ones")
        taba_sb = const.tile([P, S], F16, tag="taba")
        tabb_sb = const.tile([P, S], F16, tag="tabb")
        mask_sb = const.tile([P, NH_LOC, STILE], F16, tag="mask")
        bq_sb = const.tile([P, NH_LOC], F32, tag="bq")
        bk_sb = const.tile([P, 1], F32, tag="bk")
        eps_sb = const.tile([P, 1], F32, tag="eps")
        nc.vector.memset(eps_sb[:], EPS)
        lnc0_sb = const.tile([P, 1], F32, tag="lnc0")
        nc.vector.memset(lnc0_sb[:], float(np.log(C0)))
        expb_sb = const.tile([P, 1], F32, tag="expb")
        nc.vector.memset(expb_sb[:], EXPB)
        tiny_sb = const.tile([P, 1], F32, tag="tiny")
        nc.vector.memset(tiny_sb[:], 1e-20)

        # persistent activation tensors
        qhat = const.tile([P, NH_LOC, S], F16, tag="qhat")   # [c, h, s]
        khat = const.tile([P, S], F16, tag="khat")           # [c, s], pre-scaled
        vsb = const.tile([P, NJB, DK], F16, tag="v")         # [s%128, block, c]
        aon = const.tile([P, NH_LOC, S], F16, tag="aon")     # [c, h, i]

        # ------- Phase 1: projections + RMS + RoPE, fused per (output, s-tile) -------
        with ExitStack() as ph1:
            xp = ph1.enter_context(tc.tile_pool(name="xp", bufs=2))
            t1p = ph1.enter_context(tc.tile_pool(name="t1p", bufs=6))
            rp = ph1.enter_context(tc.tile_pool(name="rp", bufs=4))
            ps1 = ph1.enter_context(tc.tile_pool(name="ps1", bufs=3, space="PSUM"))
            plsp = ph1.enter_context(tc.tile_pool(name="plsp", bufs=2, space="PSUM"))

            # startup-critical loads first, split into chunk groups so the
            # first projection matmuls stream behind the arrivals
            xt0 = xp.tile([P, NC, STILE], F16, tag="xt", name="xt0")
            nc.sync.dma_start(wq_sb[:, 0:1, :], wq[:, 0:1, :])
            nc.sync.dma_start(xt0[:, 0:1, :], xt4[0, :, 0:1, :])
            for g4 in ((1, 4), (4, 8), (8, 12), (12, 16)):
                sl = slice(*g4)
                nc.sync.dma_start(wq_sb[:, sl, :], wq[:, sl, :])
                nc.sync.dma_start(xt0[:, sl, :], xt4[0, :, sl, :])
            nc.sync.dma_start(bq_sb[:], bq[:])
            nc.sync.dma_start(winvq_sb[:], winvq[:])
            nc.sync.dma_start(taba_sb[:], taba[:])
            nc.sync.dma_start(tabb_sb[:], tabb[:])
            # non-startup-critical bulk loads ride the idle gpsimd DGE queue.
            # The dummy copy reads xt0, so these transfers only start after
            # the startup-critical xt0 load is done and don't steal its
            # HBM bandwidth.
            dummy = const.tile([1, 1], F16, tag="dummy")
            nc.gpsimd.tensor_copy(dummy[:], xt0[0:1, NC - 1, 0:1])
            nc.gpsimd.dma_start(wv_sb[:], wv[:])
            nc.gpsimd.dma_start(wk_sb[:], wk[:])
            nc.gpsimd.dma_start(winvk_sb[:], winvk[:])
            nc.gpsimd.dma_start(bk_sb[:], bk[:])
            nc.gpsimd.dma_start(ones_sb[:], ones2d[:])
            nc.gpsimd.dma_start(mask_sb[:], maskt[:])
            nc.gpsimd.dma_start(wo_sb[:], wo[:])

            for st in range(NST):
                if st == 0:
                    xt = xt0
                else:
                    xt = xp.tile([P, NC, STILE], F16, tag="xt")
                    nc.sync.dma_start(xt[:], xt4[st])
                ssl = bass.ts(st, STILE)

                for oi in list(range(NH_LOC)) + ["v", "k"]:
                    if oi == "v":
                        # v: output [s-block=128, c=128], 4 s-blocks per s-tile
                        ptv = ps1.tile([P, STILE], F32, tag="proj", name="ptv")
                        for sb in range(4):
                            for ch in range(NC):
                                nc.tensor.matmul(ptv[:, bass.ts(sb, DK)],
                                                 xt[:, ch, bass.ts(sb, P)], wv_sb[:, ch, :],
                                                 start=(ch == 0), stop=(ch == NC - 1))
                        for sb in range(4):
                            nc.vector.tensor_copy(vsb[:, st * 4 + sb, :], ptv[:, bass.ts(sb, DK)])
                        continue
                    is_q = oi != "k"
                    pt = ps1.tile([P, STILE], F32, tag="proj")
                    for ch in range(NC):
                        lw = wq_sb[:, ch, bass.ts(oi, DK)] if is_q else wk_sb[:, ch, :]
                        nc.tensor.matmul(pt[:], lw, xt[:, ch, :],
                                         start=(ch == 0), stop=(ch == NC - 1))
                    bias_ap = bq_sb[:, oi:oi + 1] if is_q else bk_sb[:, 0:1]
                    qf = t1p.tile([P, STILE], F16, tag="qf")
                    nc.scalar.activation(qf[:], pt[:], AF.Identity, bias=bias_ap)

                    # sumsq broadcast to all partitions via replicated stationary
                    sq = t1p.tile([P, STILE], F16, tag="sq")
                    nc.vector.tensor_mul(sq[:], qf[:], qf[:])
                    pls = plsp.tile([P, STILE], F32, tag="pls")
                    nc.tensor.matmul(pls[:], winvq_sb[:] if is_q else winvk_sb[:],
                                     sq[:], start=True, stop=True)
                    t1 = t1p.tile([P, STILE], F32, tag="t1")
                    nc.scalar.activation(t1[:], pls[:], AF.Ln,
                                         scale=1.0 / DK, bias=eps_sb[:, 0:1])
                    # q: 1/rms; k: c0/rms (bias = ln(c0))
                    rrb = t1p.tile([P, STILE], F16, tag="rrb")
                    if is_q:
                        nc.scalar.activation(rrb[:], t1[:], AF.Exp, scale=-0.5)
                    else:
                        nc.scalar.activation(rrb[:], t1[:], AF.Exp, scale=-0.5,
                                             bias=lnc0_sb[:, 0:1])

                    # RoPE on de-interleaved halves: rt = qf*[cos;cos] + shift(qf)*tabb
                    ta = rp.tile([P, STILE], F16, tag="ta")
                    nc.vector.tensor_mul(ta[:], qf[:], taba_sb[:, ssl])
                    tb = rp.tile([P, STILE], F16, tag="tb")
                    if USE_SHIFT:
                        nc.vector.tensor_mul(tb[0:64, :], qf[64:P, :], tabb_sb[64:P, ssl])
                        nc.vector.tensor_mul(tb[64:P, :], qf[0:64, :], tabb_sb[0:64, ssl])
                    else:
                        sw = rp.tile([P, STILE], F16, tag="sw")
                        nc.sync.dma_start(sw[0:64, :], qf[64:P, :])
                        nc.sync.dma_start(sw[64:P, :], qf[0:64, :])
                        nc.vector.tensor_mul(tb[0:64, :], sw[0:64, :], tabb_sb[64:P, ssl])
                        nc.vector.tensor_mul(tb[64:P, :], sw[64:P, :], tabb_sb[0:64, ssl])
                    rt = rp.tile([P, STILE], F16, tag="rt")
                    nc.vector.tensor_add(rt[:], ta[:], tb[:])
                    if is_q:
                        nc.vector.tensor_mul(qhat[:, oi, ssl], rt[:], rrb[:])
                    else:
                        nc.vector.tensor_mul(khat[:, ssl], rt[:], rrb[:])

        # ---------------- Phases 2+3: attention with interleaved out-proj ----
        yT_v = yT.rearrange("(eb p) s -> eb p s", p=P)
        yup = top.enter_context(tc.tile_pool(name="yup", bufs=6))
        # out-projection emitted as (eb, st) units: 4 matmuls -> copy -> DMA.
        # st-tile st is consumable once phase-2 iteration `it`==st finished.
        _ycount = [0]

        def emit_y_unit(psy_pool, copy_eng):
            u = _ycount[0]
            if u >= NJB * NST:
                return
            _ycount[0] += 1
            st, eb = divmod(u, NJB)
            yps = psy_pool.tile([P, STILE], F32, tag="y")
            for fc in range(NH_LOC):
                nc.tensor.matmul(yps[:], wo_sb[:, fc, bass.ts(eb, P)],
                                 aon[:, fc, bass.ts(st, STILE)],
                                 start=(fc == 0), stop=(fc == NH_LOC - 1))
            yu = yup.tile([P, STILE], F16, tag="yu")
            if copy_eng == "dve":
                nc.vector.tensor_copy(yu[:], yps[:])
            else:
                nc.scalar.activation(yu[:], yps[:], AF.Copy)
            nc.sync.dma_start(yT_v[eb][:, bass.ts(st, STILE)], yu[:])

        with ExitStack() as ph2:
            pp = ph2.enter_context(tc.tile_pool(name="pp", bufs=4))
            pap = ph2.enter_context(tc.tile_pool(name="pap", bufs=4))
            lp = ph2.enter_context(tc.tile_pool(name="lp", bufs=4))
            psst = ph2.enter_context(tc.tile_pool(name="psst", bufs=2, space="PSUM"))
            psao = ph2.enter_context(tc.tile_pool(name="psao", bufs=2, space="PSUM"))
            psl = ph2.enter_context(tc.tile_pool(name="psl", bufs=1, space="PSUM"))
            psyi = ph2.enter_context(tc.tile_pool(name="psyi", bufs=1, space="PSUM"))

            for it in range(NST):
                isl = bass.ts(it, STILE)
                njb = 4 * it + 4
                for h in range(NH_LOC):
                    ao_ps = psao.tile([P, STILE], F32, tag="ao")
                    # two independent accumulation chains (even/odd sub-block)
                    # halve the serial DVE dependency depth
                    pacc_e = pap.tile([P, STILE], F16, tag="pacc_e")
                    pacc_o = pap.tile([P, STILE], F16, tag="pacc_o")
                    # jb blocks processed in pairs sharing one 2-bank psum
                    # tile and ONE exp instruction. Diagonal pairs use the
                    # earlier sub-block's column range for both subs (the
                    # extra columns are real scores, masked to 0 later).
                    for g in range(njb // 2):
                        jb0 = 2 * g
                        t0 = jb0 - 4 * it
                        lo = P * t0 if t0 > 0 else 0
                        csl = slice(lo, STILE)
                        i0 = it * STILE + lo
                        w = STILE - lo
                        st2 = psst.tile([P, 2, STILE], F32, tag="st")
                        for s in range(2):
                            nc.tensor.matmul(st2[:, s, csl],
                                             khat[:, bass.ts(jb0 + s, P)],
                                             qhat[:, h, bass.ds(i0, w)],
                                             start=True, stop=True)
                        pt2 = pp.tile([P, 2, STILE], F16, tag="p")
                        nc.scalar.activation(pt2[:, :, csl], st2[:, :, csl],
                                             AF.Exp, bias=expb_sb[:, 0:1])
                        for s in range(2):
                            jb = jb0 + s
                            t = jb - 4 * it
                            if t >= 0:
                                nc.vector.tensor_mul(pt2[:, s, csl], pt2[:, s, csl],
                                                     mask_sb[:, t, csl])
                            nc.tensor.matmul(ao_ps[:, csl], vsb[:, jb, :],
                                             pt2[:, s, csl],
                                             start=(jb == 0), stop=(jb == njb - 1))
                            pacc = pacc_e if s == 0 else pacc_o
                            if g == 0:
                                nc.vector.tensor_copy(pacc[:], pt2[:, s, :])
                            else:
                                nc.vector.tensor_add(pacc[:, csl], pacc[:, csl],
                                                     pt2[:, s, csl])
                    nc.vector.tensor_add(pacc_e[:], pacc_e[:], pacc_o[:])
                    # softmax denominator: broadcast j-sum, then 1/l = exp(-ln l)
                    pl = psl.tile([P, STILE], F32, tag="pl")
                    nc.tensor.matmul(pl[:], ones_sb[:], pacc_e[:], start=True,
                                     stop=True)
                    tl = lp.tile([P, STILE], F32, tag="tl")
                    nc.scalar.activation(tl[:], pl[:], AF.Ln, bias=tiny_sb[:, 0:1])
                    rlb = lp.tile([P, STILE], F32, tag="rlb")
                    nc.scalar.activation(rlb[:], tl[:], AF.Exp, scale=-1.0)
                    nc.vector.tensor_mul(aon[:, h, isl], ao_ps[:], rlb[:])
                    # fill phase-2's PE slack with ready out-proj units
                    for _ in range(it):
                        emit_y_unit(psyi, "dve")

        # ---------------- Phase 3: remaining out-projection units ----------
        with ExitStack() as ph3:
            psy = ph3.enter_context(tc.tile_pool(name="psy", bufs=6, space="PSUM"))
            u = 0
            while _ycount[0] < NJB * NST:
                emit_y_unit(psy, "dve" if u % 2 == 0 else "act")
                u += 1

    if split_waits:
        _split_excess_waits(nc)
    return nc


_PERM = np.concatenate([np.arange(0, DK, 2), np.arange(1, DK, 2)])  # de-interleave


def _prep_inputs(x, Wq, bq, Wk, bk, Wv, bv, Wo, bo, q_norm_w, k_norm_w):
    """Build the 8 per-core input maps. Core c -> (b = c // 4, g = c % 4)."""
    def f16(a):
        return np.ascontiguousarray(a, dtype=_F16)

    wq_p = q_norm_w[_PERM].astype(np.float32)
    wk_p = k_norm_w[_PERM].astype(np.float32)
    with np.errstate(divide="ignore"):
        winvq = np.where(wq_p != 0, 1.0 / np.maximum(wq_p * wq_p, 1e-30), 0.0)
        winvk = np.where(wk_p != 0, 1.0 / np.maximum(wk_p * wk_p, 1e-30), 0.0)

    inv_freq = 1.0 / (10000.0 ** (np.arange(0, DK, 2, dtype=np.float32) / np.float32(DK)))
    freqs = np.arange(S, dtype=np.float32)[:, None] * inv_freq[None, :]
    cosT = np.cos(freqs).T.astype(np.float32)  # [64, S]
    sinT = np.sin(freqs).T.astype(np.float32)
    taba = np.vstack([cosT, cosT])             # [128, S]
    tabb = np.vstack([sinT, -sinT])            # shifted-base layout

    pj = np.arange(P)[:, None, None]
    tt = np.arange(NH_LOC)[None, :, None]
    fi = np.arange(STILE)[None, None, :]
    maskt = ((P * tt + pj) <= fi).astype(np.float32)  # [128, 4, 512]

    xt4_b = []
    for b in range(2):
        xt = x[b].T.astype(np.float32)  # [d, s]
        xt4_b.append(f16(xt.reshape(NC, P, NST, STILE).transpose(2, 1, 0, 3)))

    winvq_rep = f16(np.tile(winvq[:, None], (1, P)))
    winvk_rep = f16(np.tile(winvk[:, None], (1, P)))

    in_maps = []
    for core in range(8):
        b, g = divmod(core, NH_LOC)
        hsl = slice(g * NH_LOC * DK, (g + 1) * NH_LOC * DK)
        ksl = slice(g * DK, (g + 1) * DK)

        wq_blk = Wq[hsl].astype(np.float32).copy()  # [512, d]
        # per-head de-interleave permutation + fold q_norm_w
        wq_blk = wq_blk.reshape(NH_LOC, DK, D)[:, _PERM, :] * wq_p[None, :, None]
        wq_t = wq_blk.reshape(NH_LOC * DK, D).T.reshape(NC, P, NH_LOC * DK)
        wq_t = wq_t.transpose(1, 0, 2)  # [P, NC, M]

        wk_blk = Wk[ksl].astype(np.float32)[_PERM, :] * wk_p[:, None]
        wk_t = wk_blk.T.reshape(NC, P, DK).transpose(1, 0, 2)
        wv_t = Wv[ksl].astype(np.float32).T.reshape(NC, P, DK).transpose(1, 0, 2)
        wo_t = Wo[:, hsl].astype(np.float32).T.reshape(NH_LOC, P, D).transpose(1, 0, 2)

        bq_blk = bq[hsl].astype(np.float32).reshape(NH_LOC, DK)[:, _PERM].T.copy()
        bk_blk = bk[ksl].astype(np.float32)[_PERM][:, None].copy()

        in_maps.append({
            "xt4": xt4_b[b],
            "wq": f16(wq_t), "wk": f16(wk_t), "wv": f16(wv_t), "wo": f16(wo_t),
            "winvq": winvq_rep, "winvk": winvk_rep,
            "ones2d": np.ones((P, P), _F16),
            "taba": f16(taba), "tabb": f16(tabb),
            "maskt": f16(maskt),
            "bq": np.ascontiguousarray(bq_blk), "bk": bk_blk,
        })
    return in_maps


_CACHED = {}


def _get_program():
    if "nc" not in _CACHED:
        _CACHED["nc"] = _build_program()
    return _CACHED["nc"]


def kernel(x, Wq, bq, Wk, bk, Wv, bv, Wo, bo, q_norm_w, k_norm_w, _trace=False, _tmpdir=None):
    x = np.asarray(x, np.float32)
    args = [np.asarray(a, np.float32) for a in
            (Wq, bq, Wk, bk, Wv, bv, Wo, bo, q_norm_w, k_norm_w)]
    Wq, bq, Wk, bk, Wv, bv, Wo, bo, q_norm_w, k_norm_w = args

    nc = _get_program()
    in_maps = _prep_inputs(x, Wq, bq, Wk, bk, Wv, bv, Wo, bo, q_norm_w, k_norm_w)
    res = run_bass_kernel_spmd(nc, in_maps, list(range(8)), trace=_trace, tmpdir=_tmpdir)

    out = np.zeros((2, S, D), np.float32)
    for core in range(8):
        b = core // 4
        out[b] += res.results[core]["yT"].astype(np.float32).T
    out += bo[None, None, :]
    # v-bias enters only via softmax-weighted average (weights sum to 1):
    if np.any(bv):
        out += (np.repeat(bv.reshape(4, DK), 4, axis=0).reshape(D) @ Wo.T)[None, None, :]
    kernel._last_result = res
    return out


# revision 34
# speedup vs baseline: 1.1293x; 1.0105x over previous
"""TRN2 Bass kernel for GQA MultiHeadAttention (B=2, S=2048, D=2048, 16 q-heads,
4 kv-heads, d_k=128) with QK-RMSNorm + interleaved RoPE + causal softmax + out-proj.

Sharding: 8 cores = (batch b in {0,1}) x (kv-head group g in {0..3}).
Each core computes its 4 q-heads' attention for its batch and a partial
out-projection y.T = Wo_g @ attn_out_g.T  [2048(e) x 2048(s)].
Host sums the 4 partials per batch (fp16) and adds bo / the bv term.

Key design points (v2):
- fp16 activations/weights end to end (same PE/DVE rate as bf16, 4x mantissa).
- No vector reciprocals anywhere: 1/x and 1/sqrt(x) computed on the scalar
  engine as exp(-ln x) / exp(-0.5 ln x); the whole program lives in the
  natural_log_exp activation table (exp/ln/identity/square) - zero mid-kernel
  ACT table loads.
- RMS-norm sums use replicated-column stationary matmuls (W[c,r]=1/w[c]^2),
  so the per-position sumsq lands broadcast across all 128 partitions and the
  normalization scale is applied as a plain elementwise multiply.
- K is pre-scaled by c0/rms_k in phase 1, so the phase-2 softmax exp uses
  immediate scale/bias (exp(s - 5); the -5 guards fp16 overflow and cancels
  in normalization).
- Softmax denominator: P tiles accumulated on DVE (fp16), then one
  ones-stationary matmul broadcasts the j-sum to all partitions; 1/l via
  ACT exp(-ln).
- RoPE half-swap done by DVE reads at shifted partition bases (no DMA).
- No softmax max-subtraction: RMSNorm bounds |score| <= sqrt(128)=11.3, and
  the -5 exp bias keeps exp(s-5) <= e^6.3 well inside fp16 range.
"""
import sys
import numpy as np

sys.path.insert(0, "/opt/trn_rl_repo")

import concourse.bass as bass  # noqa: E402
import concourse.tile as tile  # noqa: E402
from concourse import mybir  # noqa: E402
from concourse.bass_utils import run_bass_kernel_spmd  # noqa: E402

F32 = mybir.dt.float32
F16 = mybir.dt.float16
BF16 = mybir.dt.bfloat16
AF = mybir.ActivationFunctionType

P = 128
S = 2048
D = 2048
DK = 128
NH_LOC = 4          # q heads per core
NC = D // P         # 16 contraction chunks
NST = 4             # s-tiles of 512
STILE = 512
NJB = S // P        # 16 j/s blocks of 128
EPS = 1e-8
C0 = 1.0 / np.sqrt(DK)
EXPB = -5.0         # softmax exp bias (cancels in normalization)

USE_SHIFT = True    # DVE partition-base-shifted reads for the RoPE half swap
_F16 = np.float16


_NO_SPLIT_OPCODES = {"UnconditionalBranch", "Call", "RegisterMove", "EventSemaphore"}


def _split_excess_waits(nc):
    """Walrus codegen allows only 1 sync wait per instruction struct; Tile
    can emit more. Move excess waits onto same-engine NoOps inserted before."""
    import bass_rust
    counter = [0]
    for fn in nc.m.functions:
        for blk in fn.blocks:
            out = []
            changed = False
            for inst in blk.instructions:
                si = inst.sync_info
                limit = 1
                if (si is not None and len(si.on_wait) > limit
                        and inst.opcode not in _NO_SPLIT_OPCODES):
                    waits = list(si.on_wait)
                    for w in waits[:-limit]:
                        counter[0] += 1
                        nop = bass_rust.InstNoOp(
                            name=f"I-wsplit-{counter[0]}", ins=[], outs=[])
                        nop.engine = inst.engine
                        nop.sync_info = mybir.SyncInfo(on_wait=[w], on_update=[])
                        out.append(nop)
                    inst.sync_info = mybir.SyncInfo(
                        on_wait=waits[-limit:], on_update=list(si.on_update))
                    changed = True
                out.append(inst)
            if changed:
                blk.instructions = out
    return counter[0]


def _build_program(split_waits=True):
    nc = bass.Bass()

    def inp(name, shape, dt):
        return nc.declare_dram_parameter(name, list(shape), dt, isOutput=False)

    # all partition-major so every load is 128 big contiguous descriptors
    xt4 = inp("xt4", (NST, P, NC, STILE), F16)
    wq = inp("wq", (NH_LOC, P, NC, DK), F16)  # head-major: head 0 lands first
    wk = inp("wk", (P, NC, DK), F16)
    wv = inp("wv", (P, NC, DK), F16)
    wo = inp("wo", (P, NH_LOC, D), F16)
    winvq = inp("winvq", (P, P), F16)     # replicated 1/w_q^2 columns
    winvk = inp("winvk", (P, P), F16)     # replicated 1/w_k^2 columns
    ones2d = inp("ones2d", (P, P), F16)
    taba = inp("taba", (P, S), F16)       # [cos; cos]
    tabb = inp("tabb", (P, S), F16)       # [sin; -sin] (shifted-base layout)
    maskt = inp("maskt", (P, NH_LOC, STILE), F16)
    bq = inp("bq", (P, NH_LOC), F32)
    bk = inp("bk", (P, 1), F32)
    yT = nc.declare_dram_parameter("yT", [D, S], F16, isOutput=True)

    from contextlib import ExitStack

    with tile.TileContext(nc) as tc, ExitStack() as top:
        const = top.enter_context(tc.tile_pool(name="const", bufs=1))

        wq_sb = const.tile([P, NH_LOC, NC, DK], F16, tag="wq")
        wk_sb = const.tile([P, NC, DK], F16, tag="wk")
        wv_sb = const.tile([P, NC, DK], F16, tag="wv")
        wo_sb = const.tile([P, NH_LOC, D], F16, tag="wo")
        winvq_sb = const.tile([P, P], F16, tag="winvq")
        winvk_sb = const.tile([P, P], F16, tag="winvk")
        ones_sb = const.tile([P, P], F16, tag="ones")
        taba_sb = const.tile([P, S], F16, tag="taba")
        tabb_sb = const.tile([P, S], F16, tag="tabb")
        mask_sb = const.tile([P, NH_LOC, STILE], F16, tag="mask")
        bq_sb = const.tile([P, NH_LOC], F32, tag="bq")
        bk_sb = const.tile([P, 1], F32, tag="bk")
        eps_sb = const.tile([P, 1], F32, tag="eps")
        nc.vector.memset(eps_sb[:], EPS)
        lnc0_sb = const.tile([P, 1], F32, tag="lnc0")
        nc.vector.memset(lnc0_sb[:], float(np.log(C0)))
        expb_sb = const.tile([P, 1], F32, tag="expb")
        nc.vector.memset(expb_sb[:], EXPB)
        tiny_sb = const.tile([P, 1], F32, tag="tiny")
        nc.vector.memset(tiny_sb[:], 1e-20)

        # persistent activation tensors
        qhat = const.tile([P, NH_LOC, S], F16, tag="qhat")   # [c, h, s]
        khat = const.tile([P, S], F16, tag="khat")           # [c, s], pre-scaled
        vsb = const.tile([P, NJB, DK], F16, tag="v")         # [s%128, block, c]
        aon = const.tile([P, NH_LOC, S], F16, tag="aon")     # [c, h, i]

        # ------- Phase 1: projections + RMS + RoPE, fused per (output, s-tile) -------
        with ExitStack() as ph1:
            xp = ph1.enter_context(tc.tile_pool(name="xp", bufs=2))
            t1p = ph1.enter_context(tc.tile_pool(name="t1p", bufs=6))
            rp = ph1.enter_context(tc.tile_pool(name="rp", bufs=4))
            ps1 = ph1.enter_context(tc.tile_pool(name="ps1", bufs=3, space="PSUM"))
            plsp = ph1.enter_context(tc.tile_pool(name="plsp", bufs=2, space="PSUM"))

            # startup-critical loads first: head-0 weights + x chunk groups,
            # so the first projection matmuls stream behind the arrivals
            xt0 = xp.tile([P, NC, STILE], F16, tag="xt", name="xt0")
            nc.sync.dma_start(wq_sb[:, 0, :, :], wq[0])
            nc.sync.dma_start(xt0[:, 0:4, :], xt4[0, :, 0:4, :])
            for g4 in ((4, 8), (8, 12), (12, 16)):
                sl = slice(*g4)
                nc.sync.dma_start(xt0[:, sl, :], xt4[0, :, sl, :])
            for oi in range(1, NH_LOC):
                nc.sync.dma_start(wq_sb[:, oi, :, :], wq[oi])
            nc.sync.dma_start(bq_sb[:], bq[:])
            nc.sync.dma_start(winvq_sb[:], winvq[:])
            nc.sync.dma_start(taba_sb[:], taba[:])
            nc.sync.dma_start(tabb_sb[:], tabb[:])
            # non-startup-critical bulk loads ride the idle gpsimd DGE queue.
            # The dummy copy reads xt0, so these transfers only start after
            # the startup-critical xt0 load is done and don't steal its
            # HBM bandwidth.
            dummy = const.tile([1, 1], F16, tag="dummy")
            nc.gpsimd.tensor_copy(dummy[:], xt0[0:1, NC - 1, 0:1])
            nc.gpsimd.dma_start(wv_sb[:], wv[:])
            nc.gpsimd.dma_start(wk_sb[:], wk[:])
            nc.gpsimd.dma_start(winvk_sb[:], winvk[:])
            nc.gpsimd.dma_start(bk_sb[:], bk[:])
            nc.gpsimd.dma_start(ones_sb[:], ones2d[:])
            nc.gpsimd.dma_start(mask_sb[:], maskt[:])
            nc.gpsimd.dma_start(wo_sb[:], wo[:])

            for st in range(NST):
                if st == 0:
                    xt = xt0
                else:
                    xt = xp.tile([P, NC, STILE], F16, tag="xt")
                    nc.sync.dma_start(xt[:], xt4[st])
                ssl = bass.ts(st, STILE)

                for oi in list(range(NH_LOC)) + ["v", "k"]:
                    if oi == "v":
                        # v: output [s-block=128, c=128], 4 s-blocks per s-tile
                        ptv = ps1.tile([P, STILE], F32, tag="proj", name="ptv")
                        for sb in range(4):
                            for ch in range(NC):
                                nc.tensor.matmul(ptv[:, bass.ts(sb, DK)],
                                                 xt[:, ch, bass.ts(sb, P)], wv_sb[:, ch, :],
                                                 start=(ch == 0), stop=(ch == NC - 1))
                        for sb in range(4):
                            nc.vector.tensor_copy(vsb[:, st * 4 + sb, :], ptv[:, bass.ts(sb, DK)])
                        continue
                    is_q = oi != "k"
                    pt = ps1.tile([P, STILE], F32, tag="proj")
                    for ch in range(NC):
                        lw = wq_sb[:, oi, ch, :] if is_q else wk_sb[:, ch, :]
                        nc.tensor.matmul(pt[:], lw, xt[:, ch, :],
                                         start=(ch == 0), stop=(ch == NC - 1))
                    bias_ap = bq_sb[:, oi:oi + 1] if is_q else bk_sb[:, 0:1]
                    qf = t1p.tile([P, STILE], F16, tag="qf")
                    nc.scalar.activation(qf[:], pt[:], AF.Identity, bias=bias_ap)

                    # sumsq broadcast to all partitions via replicated stationary
                    sq = t1p.tile([P, STILE], F16, tag="sq")
                    nc.vector.tensor_mul(sq[:], qf[:], qf[:])
                    pls = plsp.tile([P, STILE], F32, tag="pls")
                    nc.tensor.matmul(pls[:], winvq_sb[:] if is_q else winvk_sb[:],
                                     sq[:], start=True, stop=True)
                    t1 = t1p.tile([P, STILE], F32, tag="t1")
                    nc.scalar.activation(t1[:], pls[:], AF.Ln,
                                         scale=1.0 / DK, bias=eps_sb[:, 0:1])
                    # q: 1/rms; k: c0/rms (bias = ln(c0))
                    rrb = t1p.tile([P, STILE], F16, tag="rrb")
                    if is_q:
                        nc.scalar.activation(rrb[:], t1[:], AF.Exp, scale=-0.5)
                    else:
                        nc.scalar.activation(rrb[:], t1[:], AF.Exp, scale=-0.5,
                                             bias=lnc0_sb[:, 0:1])

                    # RoPE on de-interleaved halves: rt = qf*[cos;cos] + shift(qf)*tabb
                    ta = rp.tile([P, STILE], F16, tag="ta")
                    nc.vector.tensor_mul(ta[:], qf[:], taba_sb[:, ssl])
                    tb = rp.tile([P, STILE], F16, tag="tb")
                    if USE_SHIFT:
                        nc.vector.tensor_mul(tb[0:64, :], qf[64:P, :], tabb_sb[64:P, ssl])
                        nc.vector.tensor_mul(tb[64:P, :], qf[0:64, :], tabb_sb[0:64, ssl])
                    else:
                        sw = rp.tile([P, STILE], F16, tag="sw")
                        nc.sync.dma_start(sw[0:64, :], qf[64:P, :])
                        nc.sync.dma_start(sw[64:P, :], qf[0:64, :])
                        nc.vector.tensor_mul(tb[0:64, :], sw[0:64, :], tabb_sb[64:P, ssl])
                        nc.vector.tensor_mul(tb[64:P, :], sw[64:P, :], tabb_sb[0:64, ssl])
                    rt = rp.tile([P, STILE], F16, tag="rt")
                    nc.vector.tensor_add(rt[:], ta[:], tb[:])
                    if is_q:
                        nc.vector.tensor_mul(qhat[:, oi, ssl], rt[:], rrb[:])
                    else:
                        nc.vector.tensor_mul(khat[:, ssl], rt[:], rrb[:])

        # ---------------- Phases 2+3: attention with interleaved out-proj ----
        yT_v = yT.rearrange("(eb p) s -> eb p s", p=P)
        yup = top.enter_context(tc.tile_pool(name="yup", bufs=6))
        # out-projection emitted as (eb, st) units: 4 matmuls -> copy -> DMA.
        # st-tile st is consumable once phase-2 iteration `it`==st finished.
        _ycount = [0]

        def emit_y_unit(psy_pool, copy_eng):
            u = _ycount[0]
            if u >= NJB * NST:
                return
            _ycount[0] += 1
            st, eb = divmod(u, NJB)
            yps = psy_pool.tile([P, STILE], F32, tag="y")
            for fc in range(NH_LOC):
                nc.tensor.matmul(yps[:], wo_sb[:, fc, bass.ts(eb, P)],
                                 aon[:, fc, bass.ts(st, STILE)],
                                 start=(fc == 0), stop=(fc == NH_LOC - 1))
            yu = yup.tile([P, STILE], F16, tag="yu")
            if copy_eng == "dve":
                nc.vector.tensor_copy(yu[:], yps[:])
            else:
                nc.scalar.activation(yu[:], yps[:], AF.Copy)
            nc.sync.dma_start(yT_v[eb][:, bass.ts(st, STILE)], yu[:])

        with ExitStack() as ph2:
            pp = ph2.enter_context(tc.tile_pool(name="pp", bufs=4))
            pap = ph2.enter_context(tc.tile_pool(name="pap", bufs=4))
            lp = ph2.enter_context(tc.tile_pool(name="lp", bufs=4))
            psst = ph2.enter_context(tc.tile_pool(name="psst", bufs=2, space="PSUM"))
            psao = ph2.enter_context(tc.tile_pool(name="psao", bufs=2, space="PSUM"))
            psl = ph2.enter_context(tc.tile_pool(name="psl", bufs=1, space="PSUM"))
            psyi = ph2.enter_context(tc.tile_pool(name="psyi", bufs=1, space="PSUM"))

            for it in range(NST):
                isl = bass.ts(it, STILE)
                njb = 4 * it + 4
                for h in range(NH_LOC):
                    ao_ps = psao.tile([P, STILE], F32, tag="ao")
                    # two independent accumulation chains (even/odd sub-block)
                    # halve the serial DVE dependency depth
                    pacc_e = pap.tile([P, STILE], F16, tag="pacc_e")
                    pacc_o = pap.tile([P, STILE], F16, tag="pacc_o")
                    # jb blocks processed in pairs sharing one 2-bank psum
                    # tile and ONE exp instruction. Diagonal pairs use the
                    # earlier sub-block's column range for both subs (the
                    # extra columns are real scores, masked to 0 later).
                    for g in range(njb // 2):
                        jb0 = 2 * g
                        t0 = jb0 - 4 * it
                        lo = P * t0 if t0 > 0 else 0
                        csl = slice(lo, STILE)
                        i0 = it * STILE + lo
                        w = STILE - lo
                        st2 = psst.tile([P, 2, STILE], F32, tag="st")
                        for s in range(2):
                            nc.tensor.matmul(st2[:, s, csl],
                                             khat[:, bass.ts(jb0 + s, P)],
                                             qhat[:, h, bass.ds(i0, w)],
                                             start=True, stop=True)
                        pt2 = pp.tile([P, 2, STILE], F16, tag="p")
                        nc.scalar.activation(pt2[:, :, csl], st2[:, :, csl],
                                             AF.Exp, bias=expb_sb[:, 0:1])
                        for s in range(2):
                            jb = jb0 + s
                            t = jb - 4 * it
                            if t >= 0:
                                nc.vector.tensor_mul(pt2[:, s, csl], pt2[:, s, csl],
                                                     mask_sb[:, t, csl])
                            nc.tensor.matmul(ao_ps[:, csl], vsb[:, jb, :],
                                             pt2[:, s, csl],
                                             start=(jb == 0), stop=(jb == njb - 1))
                            pacc = pacc_e if s == 0 else pacc_o
                            if g == 0:
                                nc.vector.tensor_copy(pacc[:], pt2[:, s, :])
                            else:
                                nc.vector.tensor_add(pacc[:, csl], pacc[:, csl],
                                                     pt2[:, s, csl])
                    nc.vector.tensor_add(pacc_e[:], pacc_e[:], pacc_o[:])
                    # softmax denominator: broadcast j-sum, then 1/l = exp(-ln l)
                    pl = psl.tile([P, STILE], F32, tag="pl")
                    nc.tensor.matmul(pl[:], ones_sb[:], pacc_e[:], start=True,
                                     stop=True)
                    tl = lp.tile([P, STILE], F32, tag="tl")
                    nc.scalar.activation(tl[:], pl[:], AF.Ln, bias=tiny_sb[:, 0:1])
                    rlb = lp.tile([P, STILE], F32, tag="rlb")
                    nc.scalar.activation(rlb[:], tl[:], AF.Exp, scale=-1.0)
                    nc.vector.tensor_mul(aon[:, h, isl], ao_ps[:], rlb[:])
                    # fill phase-2's PE slack with ready out-proj units
                    for _ in range(it + (1 if it >= 2 else 0)):
                        emit_y_unit(psyi, "dve")

        # ---------------- Phase 3: remaining out-projection units ----------
        with ExitStack() as ph3:
            psy = ph3.enter_context(tc.tile_pool(name="psy", bufs=6, space="PSUM"))
            u = 0
            while _ycount[0] < NJB * NST:
                emit_y_unit(psy, "dve" if u % 2 == 0 else "act")
                u += 1

    if split_waits:
        _split_excess_waits(nc)
    return nc


_PERM = np.concatenate([np.arange(0, DK, 2), np.arange(1, DK, 2)])  # de-interleave


def _prep_inputs(x, Wq, bq, Wk, bk, Wv, bv, Wo, bo, q_norm_w, k_norm_w):
    """Build the 8 per-core input maps. Core c -> (b = c // 4, g = c % 4)."""
    def f16(a):
        return np.ascontiguousarray(a, dtype=_F16)

    wq_p = q_norm_w[_PERM].astype(np.float32)
    wk_p = k_norm_w[_PERM].astype(np.float32)
    with np.errstate(divide="ignore"):
        winvq = np.where(wq_p != 0, 1.0 / np.maximum(wq_p * wq_p, 1e-30), 0.0)
        winvk = np.where(wk_p != 0, 1.0 / np.maximum(wk_p * wk_p, 1e-30), 0.0)

    inv_freq = 1.0 / (10000.0 ** (np.arange(0, DK, 2, dtype=np.float32) / np.float32(DK)))
    freqs = np.arange(S, dtype=np.float32)[:, None] * inv_freq[None, :]
    cosT = np.cos(freqs).T.astype(np.float32)  # [64, S]
    sinT = np.sin(freqs).T.astype(np.float32)
    taba = np.vstack([cosT, cosT])             # [128, S]
    tabb = np.vstack([sinT, -sinT])            # shifted-base layout

    pj = np.arange(P)[:, None, None]
    tt = np.arange(NH_LOC)[None, :, None]
    fi = np.arange(STILE)[None, None, :]
    maskt = ((P * tt + pj) <= fi).astype(np.float32)  # [128, 4, 512]

    xt4_b = []
    for b in range(2):
        xt = x[b].T.astype(np.float32)  # [d, s]
        xt4_b.append(f16(xt.reshape(NC, P, NST, STILE).transpose(2, 1, 0, 3)))

    winvq_rep = f16(np.tile(winvq[:, None], (1, P)))
    winvk_rep = f16(np.tile(winvk[:, None], (1, P)))

    in_maps = []
    for core in range(8):
        b, g = divmod(core, NH_LOC)
        hsl = slice(g * NH_LOC * DK, (g + 1) * NH_LOC * DK)
        ksl = slice(g * DK, (g + 1) * DK)

        wq_blk = Wq[hsl].astype(np.float32).copy()  # [512, d]
        # per-head de-interleave permutation + fold q_norm_w
        wq_blk = wq_blk.reshape(NH_LOC, DK, D)[:, _PERM, :] * wq_p[None, :, None]
        # [NH, P(d-part), NC, DK]: head-major, partition-major within head
        wq_t = wq_blk.transpose(2, 0, 1).reshape(NC, P, NH_LOC, DK).transpose(2, 1, 0, 3)

        wk_blk = Wk[ksl].astype(np.float32)[_PERM, :] * wk_p[:, None]
        wk_t = wk_blk.T.reshape(NC, P, DK).transpose(1, 0, 2)
        wv_t = Wv[ksl].astype(np.float32).T.reshape(NC, P, DK).transpose(1, 0, 2)
        wo_t = Wo[:, hsl].astype(np.float32).T.reshape(NH_LOC, P, D).transpose(1, 0, 2)

        bq_blk = bq[hsl].astype(np.float32).reshape(NH_LOC, DK)[:, _PERM].T.copy()
        bk_blk = bk[ksl].astype(np.float32)[_PERM][:, None].copy()

        in_maps.append({
            "xt4": xt4_b[b],
            "wq": f16(wq_t), "wk": f16(wk_t), "wv": f16(wv_t), "wo": f16(wo_t),
            "winvq": winvq_rep, "winvk": winvk_rep,
            "ones2d": np.ones((P, P), _F16),
            "taba": f16(taba), "tabb": f16(tabb),
            "maskt": f16(maskt),
            "bq": np.ascontiguousarray(bq_blk), "bk": bk_blk,
        })
    return in_maps


_CACHED = {}


def _get_program():
    if "nc" not in _CACHED:
        _CACHED["nc"] = _build_program()
    return _CACHED["nc"]


def kernel(x, Wq, bq, Wk, bk, Wv, bv, Wo, bo, q_norm_w, k_norm_w, _trace=False, _tmpdir=None):
    x = np.asarray(x, np.float32)
    args = [np.asarray(a, np.float32) for a in
            (Wq, bq, Wk, bk, Wv, bv, Wo, bo, q_norm_w, k_norm_w)]
    Wq, bq, Wk, bk, Wv, bv, Wo, bo, q_norm_w, k_norm_w = args

    nc = _get_program()
    in_maps = _prep_inputs(x, Wq, bq, Wk, bk, Wv, bv, Wo, bo, q_norm_w, k_norm_w)
    res = run_bass_kernel_spmd(nc, in_maps, list(range(8)), trace=_trace, tmpdir=_tmpdir)

    out = np.zeros((2, S, D), np.float32)
    for core in range(8):
        b = core // 4
        out[b] += res.results[core]["yT"].astype(np.float32).T
    out += bo[None, None, :]
    # v-bias enters only via softmax-weighted average (weights sum to 1):
    if np.any(bv):
        out += (np.repeat(bv.reshape(4, DK), 4, axis=0).reshape(D) @ Wo.T)[None, None, :]
    kernel._last_result = res
    return out


# revision 36
# speedup vs baseline: 1.1333x; 1.0036x over previous
"""TRN2 Bass kernel for GQA MultiHeadAttention (B=2, S=2048, D=2048, 16 q-heads,
4 kv-heads, d_k=128) with QK-RMSNorm + interleaved RoPE + causal softmax + out-proj.

Sharding: 8 cores = (batch b in {0,1}) x (kv-head group g in {0..3}).
Each core computes its 4 q-heads' attention for its batch and a partial
out-projection y.T = Wo_g @ attn_out_g.T  [2048(e) x 2048(s)].
Host sums the 4 partials per batch (fp16) and adds bo / the bv term.

Key design points (v2):
- fp16 activations/weights end to end (same PE/DVE rate as bf16, 4x mantissa).
- No vector reciprocals anywhere: 1/x and 1/sqrt(x) computed on the scalar
  engine as exp(-ln x) / exp(-0.5 ln x); the whole program lives in the
  natural_log_exp activation table (exp/ln/identity/square) - zero mid-kernel
  ACT table loads.
- RMS-norm sums use replicated-column stationary matmuls (W[c,r]=1/w[c]^2),
  so the per-position sumsq lands broadcast across all 128 partitions and the
  normalization scale is applied as a plain elementwise multiply.
- K is pre-scaled by c0/rms_k in phase 1, so the phase-2 softmax exp uses
  immediate scale/bias (exp(s - 5); the -5 guards fp16 overflow and cancels
  in normalization).
- Softmax denominator: P tiles accumulated on DVE (fp16), then one
  ones-stationary matmul broadcasts the j-sum to all partitions; 1/l via
  ACT exp(-ln).
- RoPE half-swap done by DVE reads at shifted partition bases (no DMA).
- No softmax max-subtraction: RMSNorm bounds |score| <= sqrt(128)=11.3, and
  the -5 exp bias keeps exp(s-5) <= e^6.3 well inside fp16 range.
"""
import sys
import numpy as np

sys.path.insert(0, "/opt/trn_rl_repo")

import concourse.bass as bass  # noqa: E402
import concourse.tile as tile  # noqa: E402
from concourse import mybir  # noqa: E402
from concourse.bass_utils import run_bass_kernel_spmd  # noqa: E402

F32 = mybir.dt.float32
F16 = mybir.dt.float16
BF16 = mybir.dt.bfloat16
AF = mybir.ActivationFunctionType

P = 128
S = 2048
D = 2048
DK = 128
NH_LOC = 4          # q heads per core
NC = D // P         # 16 contraction chunks
NST = 4             # s-tiles of 512
STILE = 512
NJB = S // P        # 16 j/s blocks of 128
EPS = 1e-8
C0 = 1.0 / np.sqrt(DK)
EXPB = -5.0         # softmax exp bias (cancels in normalization)

USE_SHIFT = True    # DVE partition-base-shifted reads for the RoPE half swap
_F16 = np.float16


_NO_SPLIT_OPCODES = {"UnconditionalBranch", "Call", "RegisterMove", "EventSemaphore"}


def _split_excess_waits(nc):
    """Walrus codegen allows only 1 sync wait per instruction struct; Tile
    can emit more. Move excess waits onto same-engine NoOps inserted before."""
    import bass_rust
    counter = [0]
    for fn in nc.m.functions:
        for blk in fn.blocks:
            out = []
            changed = False
            for inst in blk.instructions:
                si = inst.sync_info
                limit = 1
                if (si is not None and len(si.on_wait) > limit
                        and inst.opcode not in _NO_SPLIT_OPCODES):
                    waits = list(si.on_wait)
                    for w in waits[:-limit]:
                        counter[0] += 1
                        nop = bass_rust.InstNoOp(
                            name=f"I-wsplit-{counter[0]}", ins=[], outs=[])
                        nop.engine = inst.engine
                        nop.sync_info = mybir.SyncInfo(on_wait=[w], on_update=[])
                        out.append(nop)
                    inst.sync_info = mybir.SyncInfo(
                        on_wait=waits[-limit:], on_update=list(si.on_update))
                    changed = True
                out.append(inst)
            if changed:
                blk.instructions = out
    return counter[0]


def _build_program(split_waits=True):
    nc = bass.Bass()

    def inp(name, shape, dt):
        return nc.declare_dram_parameter(name, list(shape), dt, isOutput=False)

    # all partition-major so every load is 128 big contiguous descriptors
    xt4 = inp("xt4", (NST, P, NC, STILE), F16)
    wq = inp("wq", (NH_LOC, P, NC, DK), F16)  # head-major: head 0 lands first
    wk = inp("wk", (P, NC, DK), F16)
    wv = inp("wv", (P, NC, DK), F16)
    wo = inp("wo", (P, NH_LOC, D), F16)
    winvq = inp("winvq", (P, P), F16)     # replicated 1/w_q^2 columns
    winvk = inp("winvk", (P, P), F16)     # replicated 1/w_k^2 columns
    ones2d = inp("ones2d", (P, P), F16)
    taba = inp("taba", (P, S), F16)       # [cos; cos]
    tabb = inp("tabb", (P, S), F16)       # [sin; -sin] (shifted-base layout)
    maskt = inp("maskt", (P, NH_LOC, STILE), F16)
    bq = inp("bq", (P, NH_LOC), F32)
    bk = inp("bk", (P, 1), F32)
    yT = nc.declare_dram_parameter("yT", [D, S], F16, isOutput=True)

    from contextlib import ExitStack

    with tile.TileContext(nc) as tc, ExitStack() as top:
        const = top.enter_context(tc.tile_pool(name="const", bufs=1))

        wq_sb = const.tile([P, NH_LOC, NC, DK], F16, tag="wq")
        wk_sb = const.tile([P, NC, DK], F16, tag="wk")
        wv_sb = const.tile([P, NC, DK], F16, tag="wv")
        wo_sb = const.tile([P, NH_LOC, D], F16, tag="wo")
        winvq_sb = const.tile([P, P], F16, tag="winvq")
        winvk_sb = const.tile([P, P], F16, tag="winvk")
        ones_sb = const.tile([P, P], F16, tag="ones")
        taba_sb = const.tile([P, S], F16, tag="taba")
        tabb_sb = const.tile([P, S], F16, tag="tabb")
        mask_sb = const.tile([P, NH_LOC, STILE], F16, tag="mask")
        bq_sb = const.tile([P, NH_LOC], F32, tag="bq")
        bk_sb = const.tile([P, 1], F32, tag="bk")
        eps_sb = const.tile([P, 1], F32, tag="eps")
        nc.vector.memset(eps_sb[:], EPS)
        lnc0_sb = const.tile([P, 1], F32, tag="lnc0")
        nc.vector.memset(lnc0_sb[:], float(np.log(C0)))
        expb_sb = const.tile([P, 1], F32, tag="expb")
        nc.vector.memset(expb_sb[:], EXPB)
        tiny_sb = const.tile([P, 1], F32, tag="tiny")
        nc.vector.memset(tiny_sb[:], 1e-20)

        # persistent activation tensors
        qhat = const.tile([P, NH_LOC, S], F16, tag="qhat")   # [c, h, s]
        khat = const.tile([P, S], F16, tag="khat")           # [c, s], pre-scaled
        vsb = const.tile([P, NJB, DK], F16, tag="v")         # [s%128, block, c]
        aon = const.tile([P, NH_LOC, S], F16, tag="aon")     # [c, h, i]

        # ------- Phase 1: projections + RMS + RoPE, fused per (output, s-tile) -------
        with ExitStack() as ph1:
            xp = ph1.enter_context(tc.tile_pool(name="xp", bufs=2))
            t1p = ph1.enter_context(tc.tile_pool(name="t1p", bufs=6))
            rp = ph1.enter_context(tc.tile_pool(name="rp", bufs=4))
            ps1 = ph1.enter_context(tc.tile_pool(name="ps1", bufs=3, space="PSUM"))
            plsp = ph1.enter_context(tc.tile_pool(name="plsp", bufs=2, space="PSUM"))

            # startup-critical loads first: head-0 weights + x chunk groups,
            # so the first projection matmuls stream behind the arrivals
            xt0 = xp.tile([P, NC, STILE], F16, tag="xt", name="xt0")
            nc.sync.dma_start(wq_sb[:, 0, :, :], wq[0])
            for g4 in ((0, 2), (2, 4), (4, 8), (8, 12), (12, 16)):
                sl = slice(*g4)
                nc.sync.dma_start(xt0[:, sl, :], xt4[0, :, sl, :])
            for oi in range(1, NH_LOC):
                nc.sync.dma_start(wq_sb[:, oi, :, :], wq[oi])
            nc.sync.dma_start(bq_sb[:], bq[:])
            nc.sync.dma_start(winvq_sb[:], winvq[:])
            nc.sync.dma_start(taba_sb[:], taba[:])
            nc.sync.dma_start(tabb_sb[:], tabb[:])
            # non-startup-critical bulk loads ride the idle gpsimd DGE queue.
            # The dummy copy reads xt0, so these transfers only start after
            # the startup-critical xt0 load is done and don't steal its
            # HBM bandwidth.
            dummy = const.tile([1, 1], F16, tag="dummy")
            nc.gpsimd.tensor_copy(dummy[:], xt0[0:1, NC - 1, 0:1])
            nc.gpsimd.dma_start(wv_sb[:], wv[:])
            nc.gpsimd.dma_start(wk_sb[:], wk[:])
            nc.gpsimd.dma_start(winvk_sb[:], winvk[:])
            nc.gpsimd.dma_start(bk_sb[:], bk[:])
            nc.gpsimd.dma_start(ones_sb[:], ones2d[:])
            nc.gpsimd.dma_start(mask_sb[:], maskt[:])
            nc.gpsimd.dma_start(wo_sb[:], wo[:])

            for st in range(NST):
                if st == 0:
                    xt = xt0
                else:
                    xt = xp.tile([P, NC, STILE], F16, tag="xt")
                    nc.sync.dma_start(xt[:], xt4[st])
                ssl = bass.ts(st, STILE)

                for oi in list(range(NH_LOC)) + ["v", "k"]:
                    if oi == "v":
                        # v: output [s-block=128, c=128], 4 s-blocks per s-tile
                        ptv = ps1.tile([P, STILE], F32, tag="proj", name="ptv")
                        for sb in range(4):
                            for ch in range(NC):
                                nc.tensor.matmul(ptv[:, bass.ts(sb, DK)],
                                                 xt[:, ch, bass.ts(sb, P)], wv_sb[:, ch, :],
                                                 start=(ch == 0), stop=(ch == NC - 1))
                        for sb in range(4):
                            nc.vector.tensor_copy(vsb[:, st * 4 + sb, :], ptv[:, bass.ts(sb, DK)])
                        continue
                    is_q = oi != "k"
                    pt = ps1.tile([P, STILE], F32, tag="proj")
                    for ch in range(NC):
                        lw = wq_sb[:, oi, ch, :] if is_q else wk_sb[:, ch, :]
                        nc.tensor.matmul(pt[:], lw, xt[:, ch, :],
                                         start=(ch == 0), stop=(ch == NC - 1))
                    bias_ap = bq_sb[:, oi:oi + 1] if is_q else bk_sb[:, 0:1]
                    qf = t1p.tile([P, STILE], F16, tag="qf")
                    nc.scalar.activation(qf[:], pt[:], AF.Identity, bias=bias_ap)

                    # sumsq broadcast to all partitions via replicated stationary
                    sq = t1p.tile([P, STILE], F16, tag="sq")
                    nc.vector.tensor_mul(sq[:], qf[:], qf[:])
                    pls = plsp.tile([P, STILE], F32, tag="pls")
                    nc.tensor.matmul(pls[:], winvq_sb[:] if is_q else winvk_sb[:],
                                     sq[:], start=True, stop=True)
                    t1 = t1p.tile([P, STILE], F32, tag="t1")
                    nc.scalar.activation(t1[:], pls[:], AF.Ln,
                                         scale=1.0 / DK, bias=eps_sb[:, 0:1])
                    # q: 1/rms; k: c0/rms (bias = ln(c0))
                    rrb = t1p.tile([P, STILE], F16, tag="rrb")
                    if is_q:
                        nc.scalar.activation(rrb[:], t1[:], AF.Exp, scale=-0.5)
                    else:
                        nc.scalar.activation(rrb[:], t1[:], AF.Exp, scale=-0.5,
                                             bias=lnc0_sb[:, 0:1])

                    # RoPE on de-interleaved halves: rt = qf*[cos;cos] + shift(qf)*tabb
                    ta = rp.tile([P, STILE], F16, tag="ta")
                    nc.vector.tensor_mul(ta[:], qf[:], taba_sb[:, ssl])
                    tb = rp.tile([P, STILE], F16, tag="tb")
                    if USE_SHIFT:
                        nc.vector.tensor_mul(tb[0:64, :], qf[64:P, :], tabb_sb[64:P, ssl])
                        nc.vector.tensor_mul(tb[64:P, :], qf[0:64, :], tabb_sb[0:64, ssl])
                    else:
                        sw = rp.tile([P, STILE], F16, tag="sw")
                        nc.sync.dma_start(sw[0:64, :], qf[64:P, :])
                        nc.sync.dma_start(sw[64:P, :], qf[0:64, :])
                        nc.vector.tensor_mul(tb[0:64, :], sw[0:64, :], tabb_sb[64:P, ssl])
                        nc.vector.tensor_mul(tb[64:P, :], sw[64:P, :], tabb_sb[0:64, ssl])
                    rt = rp.tile([P, STILE], F16, tag="rt")
                    nc.vector.tensor_add(rt[:], ta[:], tb[:])
                    if is_q:
                        nc.vector.tensor_mul(qhat[:, oi, ssl], rt[:], rrb[:])
                    else:
                        nc.vector.tensor_mul(khat[:, ssl], rt[:], rrb[:])

        # ---------------- Phases 2+3: attention with interleaved out-proj ----
        yT_v = yT.rearrange("(eb p) s -> eb p s", p=P)
        yup = top.enter_context(tc.tile_pool(name="yup", bufs=8))
        # out-projection emitted as (eb, st) units: 4 matmuls -> copy -> DMA.
        # st-tile st is consumable once phase-2 iteration `it`==st finished.
        _ycount = [0]

        def emit_y_unit(psy_pool, copy_eng):
            u = _ycount[0]
            if u >= NJB * NST:
                return
            _ycount[0] += 1
            st, eb = divmod(u, NJB)
            yps = psy_pool.tile([P, STILE], F32, tag="y")
            for fc in range(NH_LOC):
                nc.tensor.matmul(yps[:], wo_sb[:, fc, bass.ts(eb, P)],
                                 aon[:, fc, bass.ts(st, STILE)],
                                 start=(fc == 0), stop=(fc == NH_LOC - 1))
            yu = yup.tile([P, STILE], F16, tag="yu")
            if copy_eng == "dve":
                nc.vector.tensor_copy(yu[:], yps[:])
            else:
                nc.scalar.activation(yu[:], yps[:], AF.Copy)
            nc.sync.dma_start(yT_v[eb][:, bass.ts(st, STILE)], yu[:])

        with ExitStack() as ph2:
            pp = ph2.enter_context(tc.tile_pool(name="pp", bufs=4))
            pap = ph2.enter_context(tc.tile_pool(name="pap", bufs=4))
            lp = ph2.enter_context(tc.tile_pool(name="lp", bufs=4))
            psst = ph2.enter_context(tc.tile_pool(name="psst", bufs=2, space="PSUM"))
            psao = ph2.enter_context(tc.tile_pool(name="psao", bufs=2, space="PSUM"))
            psl = ph2.enter_context(tc.tile_pool(name="psl", bufs=1, space="PSUM"))
            psyi = ph2.enter_context(tc.tile_pool(name="psyi", bufs=1, space="PSUM"))

            for it in range(NST):
                isl = bass.ts(it, STILE)
                njb = 4 * it + 4
                for h in range(NH_LOC):
                    ao_ps = psao.tile([P, STILE], F32, tag="ao")
                    # two independent accumulation chains (even/odd sub-block)
                    # halve the serial DVE dependency depth
                    pacc_e = pap.tile([P, STILE], F16, tag="pacc_e")
                    pacc_o = pap.tile([P, STILE], F16, tag="pacc_o")
                    # jb blocks processed in pairs sharing one 2-bank psum
                    # tile and ONE exp instruction. Diagonal pairs use the
                    # earlier sub-block's column range for both subs (the
                    # extra columns are real scores, masked to 0 later).
                    for g in range(njb // 2):
                        jb0 = 2 * g
                        t0 = jb0 - 4 * it
                        lo = P * t0 if t0 > 0 else 0
                        csl = slice(lo, STILE)
                        i0 = it * STILE + lo
                        w = STILE - lo
                        st2 = psst.tile([P, 2, STILE], F32, tag="st")
                        for s in range(2):
                            nc.tensor.matmul(st2[:, s, csl],
                                             khat[:, bass.ts(jb0 + s, P)],
                                             qhat[:, h, bass.ds(i0, w)],
                                             start=True, stop=True)
                        pt2 = pp.tile([P, 2, STILE], F16, tag="p")
                        nc.scalar.activation(pt2[:, :, csl], st2[:, :, csl],
                                             AF.Exp, bias=expb_sb[:, 0:1])
                        for s in range(2):
                            jb = jb0 + s
                            t = jb - 4 * it
                            if t >= 0:
                                nc.vector.tensor_mul(pt2[:, s, csl], pt2[:, s, csl],
                                                     mask_sb[:, t, csl])
                            nc.tensor.matmul(ao_ps[:, csl], vsb[:, jb, :],
                                             pt2[:, s, csl],
                                             start=(jb == 0), stop=(jb == njb - 1))
                            pacc = pacc_e if s == 0 else pacc_o
                            if g == 0:
                                nc.vector.tensor_copy(pacc[:], pt2[:, s, :])
                            else:
                                nc.vector.tensor_add(pacc[:, csl], pacc[:, csl],
                                                     pt2[:, s, csl])
                    nc.vector.tensor_add(pacc_e[:], pacc_e[:], pacc_o[:])
                    # softmax denominator: broadcast j-sum, then 1/l = exp(-ln l)
                    pl = psl.tile([P, STILE], F32, tag="pl")
                    nc.tensor.matmul(pl[:], ones_sb[:], pacc_e[:], start=True,
                                     stop=True)
                    tl = lp.tile([P, STILE], F32, tag="tl")
                    nc.scalar.activation(tl[:], pl[:], AF.Ln, bias=tiny_sb[:, 0:1])
                    rlb = lp.tile([P, STILE], F32, tag="rlb")
                    nc.scalar.activation(rlb[:], tl[:], AF.Exp, scale=-1.0)
                    nc.vector.tensor_mul(aon[:, h, isl], ao_ps[:], rlb[:])
                    # fill phase-2's PE slack with ready out-proj units
                    for _ in range(it + (1 if it >= 2 else 0)):
                        emit_y_unit(psyi, "dve")

        # ---------------- Phase 3: remaining out-projection units ----------
        with ExitStack() as ph3:
            psy = ph3.enter_context(tc.tile_pool(name="psy", bufs=6, space="PSUM"))
            u = 0
            while _ycount[0] < NJB * NST:
                emit_y_unit(psy, "dve" if u % 2 == 0 else "act")
                u += 1

    if split_waits:
        _split_excess_waits(nc)
    return nc


_PERM = np.concatenate([np.arange(0, DK, 2), np.arange(1, DK, 2)])  # de-interleave


def _prep_inputs(x, Wq, bq, Wk, bk, Wv, bv, Wo, bo, q_norm_w, k_norm_w):
    """Build the 8 per-core input maps. Core c -> (b = c // 4, g = c % 4)."""
    def f16(a):
        return np.ascontiguousarray(a, dtype=_F16)

    wq_p = q_norm_w[_PERM].astype(np.float32)
    wk_p = k_norm_w[_PERM].astype(np.float32)
    with np.errstate(divide="ignore"):
        winvq = np.where(wq_p != 0, 1.0 / np.maximum(wq_p * wq_p, 1e-30), 0.0)
        winvk = np.where(wk_p != 0, 1.0 / np.maximum(wk_p * wk_p, 1e-30), 0.0)

    inv_freq = 1.0 / (10000.0 ** (np.arange(0, DK, 2, dtype=np.float32) / np.float32(DK)))
    freqs = np.arange(S, dtype=np.float32)[:, None] * inv_freq[None, :]
    cosT = np.cos(freqs).T.astype(np.float32)  # [64, S]
    sinT = np.sin(freqs).T.astype(np.float32)
    taba = np.vstack([cosT, cosT])             # [128, S]
    tabb = np.vstack([sinT, -sinT])            # shifted-base layout

    pj = np.arange(P)[:, None, None]
    tt = np.arange(NH_LOC)[None, :, None]
    fi = np.arange(STILE)[None, None, :]
    maskt = ((P * tt + pj) <= fi).astype(np.float32)  # [128, 4, 512]

    xt4_b = []
    for b in range(2):
        xt = x[b].T.astype(np.float32)  # [d, s]
        xt4_b.append(f16(xt.reshape(NC, P, NST, STILE).transpose(2, 1, 0, 3)))

    winvq_rep = f16(np.tile(winvq[:, None], (1, P)))
    winvk_rep = f16(np.tile(winvk[:, None], (1, P)))

    in_maps = []
    for core in range(8):
        b, g = divmod(core, NH_LOC)
        hsl = slice(g * NH_LOC * DK, (g + 1) * NH_LOC * DK)
        ksl = slice(g * DK, (g + 1) * DK)

        wq_blk = Wq[hsl].astype(np.float32).copy()  # [512, d]
        # per-head de-interleave permutation + fold q_norm_w
        wq_blk = wq_blk.reshape(NH_LOC, DK, D)[:, _PERM, :] * wq_p[None, :, None]
        # [NH, P(d-part), NC, DK]: head-major, partition-major within head
        wq_t = wq_blk.transpose(2, 0, 1).reshape(NC, P, NH_LOC, DK).transpose(2, 1, 0, 3)

        wk_blk = Wk[ksl].astype(np.float32)[_PERM, :] * wk_p[:, None]
        wk_t = wk_blk.T.reshape(NC, P, DK).transpose(1, 0, 2)
        wv_t = Wv[ksl].astype(np.float32).T.reshape(NC, P, DK).transpose(1, 0, 2)
        wo_t = Wo[:, hsl].astype(np.float32).T.reshape(NH_LOC, P, D).transpose(1, 0, 2)

        bq_blk = bq[hsl].astype(np.float32).reshape(NH_LOC, DK)[:, _PERM].T.copy()
        bk_blk = bk[ksl].astype(np.float32)[_PERM][:, None].copy()

        in_maps.append({
            "xt4": xt4_b[b],
            "wq": f16(wq_t), "wk": f16(wk_t), "wv": f16(wv_t), "wo": f16(wo_t),
            "winvq": winvq_rep, "winvk": winvk_rep,
            "ones2d": np.ones((P, P), _F16),
            "taba": f16(taba), "tabb": f16(tabb),
            "maskt": f16(maskt),
            "bq": np.ascontiguousarray(bq_blk), "bk": bk_blk,
        })
    return in_maps


_CACHED = {}


def _get_program():
    if "nc" not in _CACHED:
        _CACHED["nc"] = _build_program()
    return _CACHED["nc"]


def kernel(x, Wq, bq, Wk, bk, Wv, bv, Wo, bo, q_norm_w, k_norm_w, _trace=False, _tmpdir=None):
    x = np.asarray(x, np.float32)
    args = [np.asarray(a, np.float32) for a in
            (Wq, bq, Wk, bk, Wv, bv, Wo, bo, q_norm_w, k_norm_w)]
    Wq, bq, Wk, bk, Wv, bv, Wo, bo, q_norm_w, k_norm_w = args

    nc = _get_program()
    in_maps = _prep_inputs(x, Wq, bq, Wk, bk, Wv, bv, Wo, bo, q_norm_w, k_norm_w)
    res = run_bass_kernel_spmd(nc, in_maps, list(range(8)), trace=_trace, tmpdir=_tmpdir)

    out = np.zeros((2, S, D), np.float32)
    for core in range(8):
        b = core // 4
        out[b] += res.results[core]["yT"].astype(np.float32).T
    out += bo[None, None, :]
    # v-bias enters only via softmax-weighted average (weights sum to 1):
    if np.any(bv):
        out += (np.repeat(bv.reshape(4, DK), 4, axis=0).reshape(D) @ Wo.T)[None, None, :]
    kernel._last_result = res
    return out
